# revision 41
# baseline (speedup 1.0000x reference)
"""Entropy-gated multi-head attention on 8 Trainium2 NeuronCores.

Sharding: core c = b*4 + g handles batch b (of 2) and head-group g (4 of the
16 heads).  Tokens with gate==0 pass x through untouched and contribute
exactly zero k/v (zero biases), so the device only processes the compacted
active tokens (~half), with the softmax denominator corrected by the count
of inactive tokens: each inactive/padded key contributes exp(0)=1 to the
softmax sum and nothing to the numerator (v=0).

v2 pipeline (ACT-saturating):
  - all inputs fp8, host pre-shuffled to [128, kt, cols] so every input DMA
    is 128 large descriptors; issued across 4 sequencer queues.
  - QT/KT projections: fp8 DoubleRow over k-tile pairs; psum halves copied
    (Pool engine) into the [128, 2, cols] DR-32 score layout (host-permuted
    W columns).
  - scores: fp8 DR-32 per (head, key-tile): lhsT kt[32h:32h+32, 2, 128],
    62ns/tile measured (4 concurrent PE row-tiles at positions 0/32/64/96).
  - exp on ACT in 3-key-tile groups [128, 3, qn] (two rotating 3-bank psum
    buffers) -> ACT is the bottleneck engine and stays ~saturated.
  - OT = [V|1]^T PT accumulated per head over key-tile DR pairs (+1 single).
  - softmax denom: Z row + CADD (Pool) -> reciprocal (DVE) -> PE K=1 matmul
    broadcast into psum -> one STT per head builds fp8 osb (x16 scale kept
    in range); no DRAM round-trips.
  - Y: fp8 DR over the head-pair planes, psum DMA'd straight to DRAM f32;
    host divides by 256 (= WSCALE^2), sums the 4 group partials, scatters.
"""

import heapq
import math
from contextlib import ExitStack

import numpy as np
import ml_dtypes

import concourse.bass as bass
import concourse.mybir as mybir
from concourse import bacc
import concourse.tile as tile
from concourse.bass_utils import run_bass_kernel_spmd

B, S, D = 2, 2048, 1024
H, DH = 16, 64
NCORES = 8
GROUPS = NCORES // B          # head-groups per batch = 4
HC = H // GROUPS              # heads per core = 4
DC = HC * DH                  # head-group width = 256

WSCALE = 16.0                 # host scale on Wq/Wk/Wv/Wo fp8 weights
LN16 = float(np.log(16.0))

f32 = mybir.dt.float32
bf16 = mybir.dt.bfloat16
fp8 = mybir.dt.float8e4
DR = mybir.MatmulPerfMode.DoubleRow
npf8 = ml_dtypes.float8_e4m3


def _qchunks(saq):
    out = []
    o = 0
    while o < saq:
        out.append((o, min(512, saq - o)))
        o += 512
    return out


def _build(SAQ: int, SAK: int) -> bass.Bass:
    nkt = D // 128            # 8 contraction tiles for projections
    nst = SAK // 128          # key tiles (kt cols are 128-padded)
    SAQP = (SAQ + 15) // 16 * 16   # qt free stride: DR subtile stride % 16
    qch = _qchunks(SAQ)
    kch = _qchunks(SAK)
    # pt = exp(s)/16; each key (incl. padding) contributes exp(0)/16 -> Z is
    # corrected by the tokens not present on the device at all:
    CADD = float(S - SAK) / 16.0
    ESCALE = 0.125 / (WSCALE * WSCALE)

    nc = bacc.Bacc()
    xT_d = nc.dram_tensor("xT", [128, nkt, SAK], fp8, kind="ExternalInput")
    wq_d = nc.dram_tensor("wq", [128, nkt, DC], fp8, kind="ExternalInput")
    wk_d = nc.dram_tensor("wk", [128, nkt, DC], fp8, kind="ExternalInput")
    wv_d = nc.dram_tensor("wv", [128, nkt, DC], fp8, kind="ExternalInput")
    wo_d = nc.dram_tensor("wo", [128, 2, D], fp8, kind="ExternalInput")
    y_d = nc.dram_tensor("y", [SAQ, D], bf16, kind="ExternalOutput")
    # narrow-tail path: per-head unnormalized Y + raw Z rows; the softmax
    # division happens on the host (removes the whole on-device r chain
    # from the critical tail)
    yt_d = nc.dram_tensor("yt", [4, 64, D], bf16, kind="ExternalOutput")
    zt_d = nc.dram_tensor("zt", [1, 4, 64], f32, kind="ExternalOutput")

    with tile.TileContext(nc) as tc, ExitStack() as ctx:
        singles = ctx.enter_context(tc.tile_pool(name="singles", bufs=1))
        pt_pool = ctx.enter_context(tc.tile_pool(name="pt", bufs=8))
        osb_pool = ctx.enter_context(tc.tile_pool(name="osb", bufs=2))
        zscr = ctx.enter_context(tc.tile_pool(name="zscr", bufs=12,
                                              space="DRAM"))
        yo_pool = ctx.enter_context(tc.tile_pool(name="yo", bufs=3))
        z_pool = ctx.enter_context(tc.tile_pool(name="z", bufs=4))
        # PSUM (8 banks): scores own a 2x2-bank ring paced only by ACT;
        # projections and Y rotate through a separate 1-bank aux pool so
        # their slow DVE-paced releases never stall the score pipeline
        stp = ctx.enter_context(tc.tile_pool(name="stp", bufs=4, space="PSUM"))
        otp = ctx.enter_context(tc.tile_pool(name="otp", bufs=4, space="PSUM"))

        # ---- persistent SBUF ----
        xt = singles.tile([128, nkt, SAK], fp8)
        wq_sb = singles.tile([128, nkt, DC], fp8)
        wk_sb = singles.tile([128, nkt, DC], fp8)
        wv_sb = singles.tile([128, nkt, DC], fp8)
        wo_sb = singles.tile([128, 2, D], fp8)
        # DR-32 score layout: head h on partitions 32h..32h+32; dim d of head
        # h at (p = 32h + d%32, j = d//32); W cols host-permuted to match.
        qt = singles.tile([128, 2, SAQP], fp8)
        kt = singles.tile([128, 2, SAK], fp8)
        # 68 cols (V | ones | pad); dual-fp8 LDWEIGHTS subtile stride must be
        # a multiple of 16 -> cols % 4 == 0
        v_aug = singles.tile([128, nst, HC, 68], fp8)
        ebias = singles.tile([128, 1], f32)

        # ---- input DMAs: 4 issuing queues, xT split so QT can start on
        # k-tile pair 0 while later pairs stream in
        nc.sync.dma_start(xt[:, 0:2, :], xT_d[:, 0:2, :])
        nc.scalar.dma_start(wq_sb[:, :, :], wq_d[:, :, :])
        nc.gpsimd.dma_start(wk_sb[:, :, :], wk_d[:, :, :])
        nc.sync.dma_start(xt[:, 2:4, :], xT_d[:, 2:4, :])
        nc.scalar.dma_start(xt[:, 4:6, :], xT_d[:, 4:6, :])
        nc.gpsimd.dma_start(xt[:, 6:8, :], xT_d[:, 6:8, :])
        nc.sync.dma_start(wv_sb[:, :, :], wv_d[:, :, :])
        nc.scalar.dma_start(wo_sb[:, :, :], wo_d[:, :, :])

        nc.gpsimd.memset(v_aug[:, :, :, 64:65], 1.0)
        nc.gpsimd.memset(v_aug[:, :, :, 65:68], 0.0)
        nc.gpsimd.memset(ebias, -LN16)

        # ---- projections ----
        def proj_qk(dst, w_sb, m, q0, qn, act=False):
            ps = stp.tile([128, 512], f32, tag="st", name="pqk")
            for t2 in range(nkt // 2):
                nc.tensor.matmul(
                    ps[:, :qn],
                    w_sb[:, 2 * t2:2 * t2 + 2, m * 128:(m + 1) * 128],
                    xt[:, 2 * t2:2 * t2 + 2, q0:q0 + qn],
                    start=(t2 == 0), stop=(t2 == nkt // 2 - 1),
                    perf_mode=DR)
            # psum halves -> DR-32 layout (cross-partition-base copies);
            # ACT before the first exp (idle there), DVE afterwards
            eng = nc.scalar.copy if act else nc.vector.tensor_copy
            eng(dst[64 * m:64 * m + 64, 0, q0:q0 + qn], ps[0:64, :qn])
            eng(dst[64 * m:64 * m + 64, 1, q0:q0 + qn], ps[64:128, :qn])

        v_done = set()

        def proj_v(s):
            v_done.add(s)
            ps = stp.tile([128, 512], f32, tag="st", name="pv")
            for t2 in range(nkt // 2):
                nc.tensor.matmul(
                    ps[:, :DC],
                    xt[:, 2 * t2:2 * t2 + 2, s * 128:(s + 1) * 128],
                    wv_sb[:, 2 * t2:2 * t2 + 2, :],
                    start=(t2 == 0), stop=(t2 == nkt // 2 - 1),
                    perf_mode=DR)
            nc.vector.tensor_copy(
                v_aug[:, s, :, 0:64],
                ps[:, :DC].rearrange("p (h d) -> p h d", h=HC))

        # natural order: the narrow tail chunk runs LAST — its endgame
        # (chain + 4 tiny STTs + 1 small y) is far cheaper than a wide
        # chunk's, and wide chunks have enough slots to hide the previous
        # chunk's chain/STT latency behind their own score/exp stream
        qord0 = list(qch)
        # upfront: only what the very first exp needs (qt chunk0 m0 + kt k0
        # m0); everything else is pulled on demand so ACT starts early
        proj_qk(qt, wq_sb, 0, qord0[0][0], qord0[0][1], act=True)
        proj_qk(kt, wk_sb, 0, kch[0][0], kch[0][1])
        qk_done = {("q", 0, qord0[0][0]), ("k", 0, 0)}

        # lazily drained PE work.  NOTE: emission order is semantic order in
        # Tile — a consumer emitted before its producer reads stale SBUF, so
        # every score emission explicitly pulls its qt/kt producers first.
        vq = list(range(nst))
        aux = [(("q", 1, qord0[0][0]),
                lambda: proj_qk(qt, wq_sb, 1, qord0[0][0], qord0[0][1]))]
        for kc, (k0, kn) in enumerate(kch):
            if kc > 0:
                aux.append((("k", 0, kc),
                            lambda a=k0, b=kn: proj_qk(kt, wk_sb, 0, a, b)))
            aux.append((("k", 1, kc),
                        lambda a=k0, b=kn: proj_qk(kt, wk_sb, 1, a, b)))
        for (q0, qn) in qord0[1:]:
            aux.append((("q", 0, q0),
                        lambda a=q0, b=qn: proj_qk(qt, wq_sb, 0, a, b)))
            aux.append((("q", 1, q0),
                        lambda a=q0, b=qn: proj_qk(qt, wq_sb, 1, a, b)))

        def ensure_v(s):
            while s not in v_done:
                proj_v(vq.pop(0))

        def drain(k):
            for _ in range(min(k, len(aux))):
                key, fn = aux.pop(0)
                qk_done.add(key)
                fn()

        def ensure_q(m, q0):
            while ("q", m, q0) not in qk_done:
                drain(1)

        def ensure_km(m, s_hi):
            # kt chunks are 512 cols = 4 key tiles each
            for kc in range((s_hi // 4) + 1):
                while ("k", m, kc) not in qk_done:
                    drain(1)

        # ---- attention: lag-scheduled emission ----
        # Every engine queue is in-order, so an instruction emitted before
        # its cross-engine producers have RUN stalls everything behind it
        # in its queue.  Scores+exps are the clock; all other work (OT, the
        # softmax chain, osb, Y, projections) is deferred via a small event
        # list and emitted a few score-slots after its producers.
        sched = []
        seqn = [0]
        slot_i = [0]
        zq_rot = [0]
        ZQS = [nc.sync, nc.scalar, nc.gpsimd]

        def after(lag, fn):
            seqn[0] += 1
            heapq.heappush(sched, (slot_i[0] + lag, seqn[0], fn))

        def run_due(flush=False):
            while sched and (flush or sched[0][0] <= slot_i[0]):
                heapq.heappop(sched)[2]()
                if flush:
                    slot_i[0] += 1

        def tick():
            slot_i[0] += 1
            run_due()

        y_osb = {}

        def y_job(q0, qn, jt, d0, act_copy=False):
            qtn = min(128, qn - jt * 128)
            osb = y_osb[(q0, qn)]
            yo = yo_pool.tile([128, 512], bf16, tag="yo", name="yo")
            yps = stp.tile([128, 512], f32, tag="st", name="yps")
            nc.tensor.matmul(
                yps[:qtn, :512],
                osb[:, :, jt * 128:jt * 128 + qtn],
                wo_sb[:, :, d0:d0 + 512],
                start=True, stop=True, perf_mode=DR)
            if act_copy:  # endgame: ACT is idle, DVE is the serial tail
                nc.scalar.copy(yo[:qtn, :], yps[:qtn, :512])
            else:
                nc.vector.tensor_copy(yo[:qtn, :], yps[:qtn, :512])
            r0 = q0 + jt * 128
            nc.scalar.dma_start(y_d[r0:r0 + qtn, d0:d0 + 512], yo[:qtn, :])

        def mk_ot_pair(ot_ps, pt, heads, sp, qn, last):
            def fn():
                ensure_v(sp)
                ensure_v(sp + 1)
                for h in heads:
                    nc.tensor.matmul(
                        ot_ps[h][:, :qn],
                        v_aug[:, sp:sp + 2, h, :],
                        pt[h][:, sp:sp + 2, :qn],
                        start=(sp == 0), stop=last, perf_mode=DR)
            return fn

        def mk_ot_single(ot_ps, pt, heads, s, qn):
            def fn():
                ensure_v(s)
                for h in heads:
                    nc.tensor.matmul(
                        ot_ps[h][:, :qn],
                        v_aug[:, s, h, 0:68],
                        pt[h][:, s, :qn],
                        start=(s == 0), stop=True)
            return fn

        def mk_chain_all(ot_ps, rb, qn):
            # 1/(Z+CADD) for all 4 heads in one chain: Z rows -> one SBUF
            # [4, qz] tile -> one [qp, 16] DMA transpose-pack (wide DVE
            # reciprocal; single-lane [1, qn] is ~15x slower) -> one DRAM
            # bounce -> 4 partition-broadcast DMAs
            def fn():
                qz = (qn + 15) // 16 * 16
                qp = qz // 4
                zall = z_pool.tile([1, 4, 512], f32, tag="zrow", name="zall")
                if qz > qn:
                    nc.vector.memset(zall[0:1, :, qn:qz], 1.0)
                for h in range(HC):
                    nc.vector.tensor_scalar(
                        out=zall[0:1, h, :qn], in0=ot_ps[h][64:65, :qn],
                        scalar1=CADD, scalar2=None, op0=mybir.AluOpType.add)
                # sync queue (HWDGE, ~0.6us/issue; gpsimd SWDGE is ~1us);
                # chain hops are emitted before this chunk's y DMAs, and y
                # rides the scalar queue anyway
                zq = z_pool.tile([128, 16], f32, tag="zq", name="zq")
                nc.sync.dma_start(zq[:qp, :], zall[0:1, :, :qz])
                nc.vector.reciprocal(zq[:qp, :], zq[:qp, :])
                zd = zscr.tile([4, 512], f32, tag="zd", name="zd")
                nc.sync.dma_start(zd[0:4, :qz], zq[:qp, :])
                for h in range(HC):
                    r0 = (h % 2) * 64
                    dqb = nc.sync if h % 2 == 0 else nc.scalar
                    dqb.dma_start(rb[r0:r0 + 64, h // 2, :qn],
                                  zd[h:h + 1, :qn].to_broadcast((64, qn)))
            return fn

        def mk_stt(ot_ps, rb, osball, p, h, qn):
            def fn():
                r0 = (h % 2) * 64
                nc.vector.scalar_tensor_tensor(
                    out=osball[r0:r0 + 64, p, :qn],
                    in0=ot_ps[h][0:64, :qn],
                    scalar=1.0,
                    in1=rb[r0:r0 + 64, p, :qn],
                    op0=mybir.AluOpType.mult,
                    op1=mybir.AluOpType.mult)
            return fn

        for ci, (q0, qn) in enumerate(qord0):
            # all 4 heads at once: score matmuls rotate PE row-tile
            # positions 0/32/64/96, which is what lets LDWEIGHTS pipeline
            # (measured 62 ns/score at 4-deep rotation vs 425 ns blocked)
            tail = nst * qn <= 512
            pt = {}
            for h in range(HC):
                pt[h] = pt_pool.tile([128, nst, 512], fp8, tag="pt",
                                     name="pt")
            if not tail:
                osball = osb_pool.tile([128, 2, 512], fp8, tag="osb",
                                       name="osball")
                y_osb[(q0, qn)] = osball
                ot_ps = {}
                for h in range(HC):
                    ot_ps[h] = otp.tile([68, 512], f32, tag="ot",
                                        name="otps")
                rb = z_pool.tile([128, 2, 512], f32, tag="rb", name="rb")
            if tail:
                # everything rides the fast-recycling "st" ring: no OT-ring
                # WAR on the previous chunk's chain, no on-device r at all
                run_due()
                for h in range(HC):
                    ensure_q(h // 2, q0)
                    ensure_km(h // 2, nst - 1)
                drain(len(aux))
                for s in range(nst):
                    ensure_v(s)
                zrows = z_pool.tile([1, 4, 64], f32, tag="ztl", name="zrows")
                otu = z_pool.tile([128, 512], fp8, tag="otu", name="otu")
                for h in range(HC):
                    st = stp.tile([128, nst, qn], f32, tag="st", name="st")
                    for s in range(nst):
                        nc.tensor.matmul(
                            st[:, s, :qn],
                            kt[32 * h:32 * h + 32, :,
                               s * 128:(s + 1) * 128],
                            qt[32 * h:32 * h + 32, :, q0:q0 + qn],
                            start=True, stop=True, perf_mode=DR,
                            tile_position=(32 * h, 0))
                    nc.scalar.activation(
                        pt[h][:, :, :qn], st[:, :, :qn],
                        mybir.ActivationFunctionType.Exp,
                        scale=ESCALE, bias=ebias[:, :])
                    tick()
                # pull the previous chunk's deferred y here (its chain has
                # had time to land) so it doesn't split the tail's OT/yh run
                for _ in range(8):
                    tick()
                for h in range(HC):
                    r0 = (h % 2) * 64
                    otps = stp.tile([128, 512], f32, tag="st", name="otps")
                    for sp in range(0, nst - 1, 2):
                        nc.tensor.matmul(
                            otps[0:68, :qn],
                            v_aug[:, sp:sp + 2, h, :],
                            pt[h][:, sp:sp + 2, :qn],
                            start=(sp == 0), stop=(sp + 2 == nst),
                            perf_mode=DR)
                    if nst % 2 == 1:
                        nc.tensor.matmul(
                            otps[0:68, :qn],
                            v_aug[:, nst - 1, h, 0:68],
                            pt[h][:, nst - 1, :qn],
                            start=(nst == 1), stop=True)
                    nc.vector.tensor_copy(zrows[0:1, h, :qn],
                                          otps[64:65, :qn])
                    # unnormalized OT can exceed fp8 range: store OT/64
                    nc.vector.tensor_scalar(
                        out=otu[r0:r0 + 64, :qn], in0=otps[0:64, :qn],
                        scalar1=1.0 / 64.0, scalar2=None,
                        op0=mybir.AluOpType.mult)
                    for di, d0 in enumerate((0, 512)):
                        yh = stp.tile([128, 512], f32, tag="st", name="yh")
                        nc.tensor.matmul(
                            yh[:qn, :512],
                            otu[r0:r0 + 64, :qn],
                            wo_sb[r0:r0 + 64, h // 2, d0:d0 + 512],
                            start=True, stop=True)
                        yho = yo_pool.tile([128, 512], bf16, tag="yo",
                                           name="yho")
                        nc.scalar.copy(yho[:qn, :], yh[:qn, :512])
                        nc.scalar.dma_start(
                            yt_d[h, 0:qn, d0:d0 + 512], yho[:qn, :])
                nc.sync.dma_start(zt_d[0:1, :, :], zrows[0:1, :, :])
                run_due(flush=True)
                continue
            else:
                for s in range(nst):
                    run_due()
                    drain(1)
                    sts = {}
                    for h in range(HC):
                        if h % 2 == 0:
                            ensure_q(h // 2, q0)
                            ensure_km(h // 2, s)
                        sts[h] = stp.tile([128, 512], f32, tag="st",
                                          name="st")
                        nc.tensor.matmul(
                            sts[h][:, :qn],
                            kt[32 * h:32 * h + 32, :,
                               s * 128:(s + 1) * 128],
                            qt[32 * h:32 * h + 32, :, q0:q0 + qn],
                            start=True, stop=True, perf_mode=DR,
                            tile_position=(32 * h, 0))
                    for h in range(HC):
                        nc.scalar.activation(
                            pt[h][:, s, :qn], sts[h][:, :qn],
                            mybir.ActivationFunctionType.Exp,
                            scale=ESCALE, bias=ebias[:, :])
                    if s % 2 == 1:
                        after(2, mk_ot_pair(ot_ps, pt, range(HC), s - 1, qn,
                                            s + 1 == nst))
                    tick()
            # emission-order constraints (in-order queues + pool-slot WAR
            # deps bind at emission): single-OT < chain < STT, and STT(c)
            # before chunk c+1's first OT; all Y(c) before STT(c+2)
            if tail:
                for j in range(0, nst - 1, 2):
                    after(1, mk_ot_pair(ot_ps, pt, range(HC), j, qn,
                                        j + 2 == nst))
            if nst % 2 == 1:
                after(1, mk_ot_single(ot_ps, pt, range(HC), nst - 1, qn))
            after(1, mk_chain_all(ot_ps, rb, qn))
            for h in range(HC):
                after(2, mk_stt(ot_ps, rb, osball, h // 2, h, qn))
            last_chunk = ci == len(qord0) - 1
            for jt in range((qn + 127) // 128):
                for hi, d0 in enumerate((0, 512)):
                    after(4 + jt + hi,
                          lambda a=q0, b=qn, j=jt, d=d0, L=last_chunk:
                          y_job(a, b, j, d, L))
        drain(len(aux))
        while vq:
            proj_v(vq.pop(0))
        run_due(flush=True)
    nc.compile()
    return nc


_nc_cache: dict = {}


def _get_nc(SAQ, SAK):
    key = (SAQ, SAK)
    if key not in _nc_cache:
        _nc_cache[key] = _build(SAQ, SAK)
    return _nc_cache[key]


def _score_perm():
    """W column permutation (within each head-group's 256 cols) for the
    DR-32 layout: psum partitions [0:64] of the m-half are the j=0 subtile
    (dims 0..31 of heads 2m, 2m+1), [64:128] the j=1 subtile (dims 32..63),
    so head h sits at qt[32h:32h+32, j, :] with dim d = j*32 + p%32."""
    perm = np.empty(2 * 128, np.int64)
    for m in range(2):
        for p in range(128):
            h = 2 * m + (p % 64) // 32
            d = (p // 64) * 32 + (p % 32)
            perm[m * 128 + p] = h * 64 + d
    return perm


def _shuf(a):
    """[D, C] -> [128, D//128, C] host-contiguous fp8 (partition-major)."""
    Dd, C = a.shape
    return np.ascontiguousarray(
        a.reshape(Dd // 128, 128, C).transpose(1, 0, 2)).astype(npf8)


def _reference_fallback(x, gate, Wq, bq, Wk, bk, Wv, bv, Wo, bo):
    g = gate.astype(x.dtype)[..., None]
    q = (x @ Wq + bq) * g
    k = (x @ Wk + bk) * g
    v = (x @ Wv + bv) * g

    def split(t):
        return t.reshape(B, S, H, DH).transpose(0, 2, 1, 3)

    q, k, v = split(q), split(k), split(v)
    sc = np.einsum('bhqd,bhkd->bhqk', q, k) / np.float32(math.sqrt(DH))
    sc = sc - sc.max(axis=-1, keepdims=True)
    e = np.exp(sc)
    attn = e / e.sum(axis=-1, keepdims=True)
    out = np.einsum('bhqk,bhkd->bhqd', attn, v)
    out = out.transpose(0, 2, 1, 3).reshape(B, S, D)
    out = out @ Wo + bo
    return (x * (1.0 - g) + out * g).astype(np.float32)


def kernel(x, gate, Wq, bq, Wk, bk, Wv, bv, Wo, bo, _profile=None):
    x = np.asarray(x, np.float32)
    gate = np.asarray(gate)
    args = dict(x=x, gate=gate, Wq=np.asarray(Wq, np.float32),
                bq=np.asarray(bq, np.float32), Wk=np.asarray(Wk, np.float32),
                bk=np.asarray(bk, np.float32), Wv=np.asarray(Wv, np.float32),
                bv=np.asarray(bv, np.float32), Wo=np.asarray(Wo, np.float32),
                bo=np.asarray(bo, np.float32))

    idxs = [np.nonzero(gate[b])[0] for b in range(B)]
    n_act = [len(i) for i in idxs]
    # the compaction trick needs zero q/k/v biases and at least one active
    # and one inactive token per batch; otherwise fall back to exact numpy
    if (any(np.abs(args[k]).max() > 0 for k in ("bq", "bk", "bv"))
            or min(n_act) == 0 or max(n_act) == S):
        return _reference_fallback(**args)

    SAQ = max(n_act)
    SAK = ((SAQ + 127) // 128) * 128
    perm = _score_perm()

    in_maps = []
    for b in range(B):
        xa = np.zeros((SAK, D), np.float32)
        xa[:n_act[b]] = x[b, idxs[b]]
        xT = _shuf(np.ascontiguousarray(xa.T))
        for g in range(GROUPS):
            cs = slice(g * DC, (g + 1) * DC)
            in_maps.append({
                "xT": xT,
                "wq": _shuf(args["Wq"][:, cs][:, perm] * WSCALE),
                "wk": _shuf(args["Wk"][:, cs][:, perm] * WSCALE),
                "wv": _shuf(args["Wv"][:, cs] * WSCALE),
                "wo": np.ascontiguousarray(
                    (args["Wo"][cs, :] * WSCALE).reshape(2, 128, D)
                    .transpose(1, 0, 2)).astype(npf8),
            })

    nc = _get_nc(SAQ, SAK)
    kw = dict(_profile) if _profile else {}
    kw.pop("result", None)
    res = run_bass_kernel_spmd(nc, in_maps, core_ids=list(range(NCORES)), **kw)
    if _profile is not None:
        _profile["result"] = res

    out = x.copy()
    inv = 1.0 / (WSCALE * WSCALE)
    nst = SAK // 128
    qch = []
    o = 0
    while o < SAQ:
        qch.append((o, min(512, SAQ - o)))
        o += 512
    tq0, tqn = qch[-1]
    has_tail = nst * tqn <= 512
    cadd = float(S - SAK) / 16.0
    for b in range(B):
        Y = np.zeros((SAQ, D), np.float32)
        for g in range(GROUPS):
            r = res.results[b * GROUPS + g]
            yv = r["y"].astype(np.float32)
            if has_tail:
                yv[tq0:] = 0.0   # device never writes tail rows of y
            Y += yv
            if has_tail:
                # narrow-tail rows: per-head unnormalized Y + raw Z; apply
                # the softmax denominator here
                yt = np.asarray(r["yt"], np.float32)    # [4, 64, D]
                zt = np.asarray(r["zt"], np.float32)[0]  # [4, 64]
                rr = 64.0 / (zt[:, :tqn] + cadd)         # [4, tqn]
                Y[tq0:tq0 + tqn] += np.einsum(
                    'hqd,hq->qd', yt[:, :tqn], rr)
        out[b, idxs[b]] = Y[:n_act[b]] * inv + args["bo"]
    return out


# revision 42
# speedup vs baseline: 1.1884x; 1.1884x over previous
"""Entropy-gated multi-head attention on 8 Trainium2 NeuronCores.

Sharding: core c = b*4 + g handles batch b (of 2) and head-group g (4 of the
16 heads).  Tokens with gate==0 pass x through untouched and contribute
exactly zero k/v (zero biases), so the device only processes the compacted
active tokens (~half), with the softmax denominator corrected by the count
of inactive tokens: each inactive/padded key contributes exp(0)=1 to the
softmax sum and nothing to the numerator (v=0).

v2 pipeline (ACT-saturating):
  - all inputs fp8, host pre-shuffled to [128, kt, cols] so every input DMA
    is 128 large descriptors; issued across 4 sequencer queues.
  - QT/KT projections: fp8 DoubleRow over k-tile pairs; psum halves copied
    (Pool engine) into the [128, 2, cols] DR-32 score layout (host-permuted
    W columns).
  - scores: fp8 DR-32 per (head, key-tile): lhsT kt[32h:32h+32, 2, 128],
    62ns/tile measured (4 concurrent PE row-tiles at positions 0/32/64/96).
  - exp on ACT in 3-key-tile groups [128, 3, qn] (two rotating 3-bank psum
    buffers) -> ACT is the bottleneck engine and stays ~saturated.
  - OT = [V|1]^T PT accumulated per head over key-tile DR pairs (+1 single).
  - softmax denom: Z row + CADD (Pool) -> reciprocal (DVE) -> PE K=1 matmul
    broadcast into psum -> one STT per head builds fp8 osb (x16 scale kept
    in range); no DRAM round-trips.
  - Y: fp8 DR over the head-pair planes, psum DMA'd straight to DRAM f32;
    host divides by 256 (= WSCALE^2), sums the 4 group partials, scatters.
"""

import heapq
import math
from contextlib import ExitStack

import numpy as np
import ml_dtypes

import concourse.bass as bass
import concourse.mybir as mybir
from concourse import bacc
import concourse.tile as tile
from concourse.bass_utils import run_bass_kernel_spmd

B, S, D = 2, 2048, 1024
H, DH = 16, 64
NCORES = 8
GROUPS = NCORES // B          # head-groups per batch = 4
HC = H // GROUPS              # heads per core = 4
DC = HC * DH                  # head-group width = 256

WSCALE = 16.0                 # host scale on Wq/Wk/Wv/Wo fp8 weights
LN16 = float(np.log(16.0))

f32 = mybir.dt.float32
bf16 = mybir.dt.bfloat16
fp8 = mybir.dt.float8e4
DR = mybir.MatmulPerfMode.DoubleRow
npf8 = ml_dtypes.float8_e4m3


def _qchunks(saq):
    out = []
    o = 0
    while o < saq:
        out.append((o, min(512, saq - o)))
        o += 512
    return out


def _build(SAQ: int, SAK: int) -> bass.Bass:
    nkt = D // 128            # 8 contraction tiles for projections
    nst = SAK // 128          # key tiles (kt cols are 128-padded)
    SAQP = (SAQ + 15) // 16 * 16   # qt free stride: DR subtile stride % 16
    qch = _qchunks(SAQ)
    kch = _qchunks(SAK)
    # pt = exp(s)/16; each key (incl. padding) contributes exp(0)/16 -> Z is
    # corrected by the tokens not present on the device at all:
    CADD = float(S - SAK) / 16.0
    ESCALE = 0.125 / (WSCALE * WSCALE)

    nc = bacc.Bacc()
    xT_d = nc.dram_tensor("xT", [128, nkt, SAK], fp8, kind="ExternalInput")
    wq_d = nc.dram_tensor("wq", [128, nkt, DC], fp8, kind="ExternalInput")
    wk_d = nc.dram_tensor("wk", [128, nkt, DC], fp8, kind="ExternalInput")
    wv_d = nc.dram_tensor("wv", [128, nkt, DC], fp8, kind="ExternalInput")
    wo_d = nc.dram_tensor("wo", [128, 2, D], fp8, kind="ExternalInput")
    y_d = nc.dram_tensor("y", [SAQ, D], bf16, kind="ExternalOutput")
    # narrow-tail path: per-head unnormalized Y + raw Z rows; the softmax
    # division happens on the host (removes the whole on-device r chain
    # from the critical tail)
    yt_d = nc.dram_tensor("yt", [4, 64, D], bf16, kind="ExternalOutput")
    zt_d = nc.dram_tensor("zt", [1, 4, 64], f32, kind="ExternalOutput")

    with tile.TileContext(nc) as tc, ExitStack() as ctx:
        singles = ctx.enter_context(tc.tile_pool(name="singles", bufs=1))
        pt_pool = ctx.enter_context(tc.tile_pool(name="pt", bufs=8))
        osb_pool = ctx.enter_context(tc.tile_pool(name="osb", bufs=2))
        zscr = ctx.enter_context(tc.tile_pool(name="zscr", bufs=12,
                                              space="DRAM"))
        yo_pool = ctx.enter_context(tc.tile_pool(name="yo", bufs=3))
        z_pool = ctx.enter_context(tc.tile_pool(name="z", bufs=4))
        # PSUM (8 banks): scores own a 2x2-bank ring paced only by ACT;
        # projections and Y rotate through a separate 1-bank aux pool so
        # their slow DVE-paced releases never stall the score pipeline
        stp = ctx.enter_context(tc.tile_pool(name="stp", bufs=4, space="PSUM"))
        otp = ctx.enter_context(tc.tile_pool(name="otp", bufs=4, space="PSUM"))

        # ---- persistent SBUF ----
        xt = singles.tile([128, nkt, SAK], fp8)
        wq_sb = singles.tile([128, nkt, DC], fp8)
        wk_sb = singles.tile([128, nkt, DC], fp8)
        wv_sb = singles.tile([128, nkt, DC], fp8)
        wo_sb = singles.tile([128, 2, D], fp8)
        # DR-32 score layout: head h on partitions 32h..32h+32; dim d of head
        # h at (p = 32h + d%32, j = d//32); W cols host-permuted to match.
        qt = singles.tile([128, 2, SAQP], fp8)
        kt = singles.tile([128, 2, SAK], fp8)
        # 68 cols (V | ones | pad); dual-fp8 LDWEIGHTS subtile stride must be
        # a multiple of 16 -> cols % 4 == 0
        v_aug = singles.tile([128, nst, HC, 68], fp8)
        ebias = singles.tile([128, 1], f32)

        # ---- input DMAs: 4 issuing queues, xT split so QT can start on
        # k-tile pair 0 while later pairs stream in
        nc.sync.dma_start(xt[:, 0:2, :], xT_d[:, 0:2, :])
        nc.scalar.dma_start(wq_sb[:, :, :], wq_d[:, :, :])
        nc.gpsimd.dma_start(wk_sb[:, :, :], wk_d[:, :, :])
        nc.sync.dma_start(xt[:, 2:4, :], xT_d[:, 2:4, :])
        nc.scalar.dma_start(xt[:, 4:6, :], xT_d[:, 4:6, :])
        nc.gpsimd.dma_start(xt[:, 6:8, :], xT_d[:, 6:8, :])
        nc.sync.dma_start(wv_sb[:, :, :], wv_d[:, :, :])
        nc.scalar.dma_start(wo_sb[:, :, :], wo_d[:, :, :])

        nc.gpsimd.memset(v_aug[:, :, :, 64:65], 1.0)
        nc.gpsimd.memset(v_aug[:, :, :, 65:68], 0.0)
        nc.gpsimd.memset(ebias, -LN16)

        # ---- projections ----
        def proj_qk(dst, w_sb, m, q0, qn, act=False):
            ps = stp.tile([128, 512], f32, tag="st", name="pqk")
            for t2 in range(nkt // 2):
                nc.tensor.matmul(
                    ps[:, :qn],
                    w_sb[:, 2 * t2:2 * t2 + 2, m * 128:(m + 1) * 128],
                    xt[:, 2 * t2:2 * t2 + 2, q0:q0 + qn],
                    start=(t2 == 0), stop=(t2 == nkt // 2 - 1),
                    perf_mode=DR)
            # psum halves -> DR-32 layout (cross-partition-base copies);
            # ACT before the first exp (idle there), DVE afterwards
            eng = nc.scalar.copy if act else nc.vector.tensor_copy
            eng(dst[64 * m:64 * m + 64, 0, q0:q0 + qn], ps[0:64, :qn])
            eng(dst[64 * m:64 * m + 64, 1, q0:q0 + qn], ps[64:128, :qn])

        v_done = set()

        def proj_v(s):
            v_done.add(s)
            ps = stp.tile([128, 512], f32, tag="st", name="pv")
            for t2 in range(nkt // 2):
                nc.tensor.matmul(
                    ps[:, :DC],
                    xt[:, 2 * t2:2 * t2 + 2, s * 128:(s + 1) * 128],
                    wv_sb[:, 2 * t2:2 * t2 + 2, :],
                    start=(t2 == 0), stop=(t2 == nkt // 2 - 1),
                    perf_mode=DR)
            nc.vector.tensor_copy(
                v_aug[:, s, :, 0:64],
                ps[:, :DC].rearrange("p (h d) -> p h d", h=HC))

        # natural order: the narrow tail chunk runs LAST — its endgame
        # (chain + 4 tiny STTs + 1 small y) is far cheaper than a wide
        # chunk's, and wide chunks have enough slots to hide the previous
        # chunk's chain/STT latency behind their own score/exp stream
        qord0 = list(qch)
        # upfront: only what the very first exp needs (qt chunk0 m0 + kt k0
        # m0); everything else is pulled on demand so ACT starts early
        proj_qk(qt, wq_sb, 0, qord0[0][0], qord0[0][1])
        proj_qk(kt, wk_sb, 0, kch[0][0], kch[0][1])
        qk_done = {("q", 0, qord0[0][0]), ("k", 0, 0)}

        # lazily drained PE work.  NOTE: emission order is semantic order in
        # Tile — a consumer emitted before its producer reads stale SBUF, so
        # every score emission explicitly pulls its qt/kt producers first.
        vq = list(range(nst))
        aux = [(("q", 1, qord0[0][0]),
                lambda: proj_qk(qt, wq_sb, 1, qord0[0][0], qord0[0][1]))]
        for kc, (k0, kn) in enumerate(kch):
            if kc > 0:
                aux.append((("k", 0, kc),
                            lambda a=k0, b=kn: proj_qk(kt, wk_sb, 0, a, b)))
            aux.append((("k", 1, kc),
                        lambda a=k0, b=kn: proj_qk(kt, wk_sb, 1, a, b)))
        for (q0, qn) in qord0[1:]:
            aux.append((("q", 0, q0),
                        lambda a=q0, b=qn: proj_qk(qt, wq_sb, 0, a, b)))
            aux.append((("q", 1, q0),
                        lambda a=q0, b=qn: proj_qk(qt, wq_sb, 1, a, b)))

        def ensure_v(s):
            while s not in v_done:
                proj_v(vq.pop(0))

        def drain(k):
            for _ in range(min(k, len(aux))):
                key, fn = aux.pop(0)
                qk_done.add(key)
                fn()

        def ensure_q(m, q0):
            while ("q", m, q0) not in qk_done:
                drain(1)

        def ensure_km(m, s_hi):
            # kt chunks are 512 cols = 4 key tiles each
            for kc in range((s_hi // 4) + 1):
                while ("k", m, kc) not in qk_done:
                    drain(1)

        # ---- attention: lag-scheduled emission ----
        # Every engine queue is in-order, so an instruction emitted before
        # its cross-engine producers have RUN stalls everything behind it
        # in its queue.  Scores+exps are the clock; all other work (OT, the
        # softmax chain, osb, Y, projections) is deferred via a small event
        # list and emitted a few score-slots after its producers.
        sched = []
        seqn = [0]
        slot_i = [0]
        zq_rot = [0]
        ZQS = [nc.sync, nc.scalar, nc.gpsimd]

        def after(lag, fn):
            seqn[0] += 1
            heapq.heappush(sched, (slot_i[0] + lag, seqn[0], fn))

        def run_due(flush=False):
            while sched and (flush or sched[0][0] <= slot_i[0]):
                heapq.heappop(sched)[2]()
                if flush:
                    slot_i[0] += 1

        def tick():
            slot_i[0] += 1
            run_due()

        y_osb = {}

        def y_job(q0, qn, jt, d0, act_copy=False):
            qtn = min(128, qn - jt * 128)
            osb = y_osb[(q0, qn)]
            yo = yo_pool.tile([128, 512], bf16, tag="yo", name="yo")
            yps = stp.tile([128, 512], f32, tag="st", name="yps")
            nc.tensor.matmul(
                yps[:qtn, :512],
                osb[:, :, jt * 128:jt * 128 + qtn],
                wo_sb[:, :, d0:d0 + 512],
                start=True, stop=True, perf_mode=DR)
            if act_copy:  # endgame: ACT is idle, DVE is the serial tail
                nc.scalar.copy(yo[:qtn, :], yps[:qtn, :512])
            else:
                nc.vector.tensor_copy(yo[:qtn, :], yps[:qtn, :512])
            r0 = q0 + jt * 128
            nc.scalar.dma_start(y_d[r0:r0 + qtn, d0:d0 + 512], yo[:qtn, :])

        def mk_ot_pair(ot_ps, pt, heads, sp, qn, last):
            def fn():
                ensure_v(sp)
                ensure_v(sp + 1)
                for h in heads:
                    nc.tensor.matmul(
                        ot_ps[h][:, :qn],
                        v_aug[:, sp:sp + 2, h, :],
                        pt[h][:, sp:sp + 2, :qn],
                        start=(sp == 0), stop=last, perf_mode=DR)
            return fn

        def mk_ot_single(ot_ps, pt, heads, s, qn):
            def fn():
                ensure_v(s)
                for h in heads:
                    nc.tensor.matmul(
                        ot_ps[h][:, :qn],
                        v_aug[:, s, h, 0:68],
                        pt[h][:, s, :qn],
                        start=(s == 0), stop=True)
            return fn

        def mk_chain_all(ot_ps, rb, qn):
            # 1/(Z+CADD) for all 4 heads in one chain: Z rows -> one SBUF
            # [4, qz] tile -> one [qp, 16] DMA transpose-pack (wide DVE
            # reciprocal; single-lane [1, qn] is ~15x slower) -> one DRAM
            # bounce -> 4 partition-broadcast DMAs
            def fn():
                qz = (qn + 15) // 16 * 16
                qp = qz // 4
                zall = z_pool.tile([1, 4, 512], f32, tag="zrow", name="zall")
                if qz > qn:
                    nc.vector.memset(zall[0:1, :, qn:qz], 1.0)
                for h in range(HC):
                    nc.vector.tensor_scalar(
                        out=zall[0:1, h, :qn], in0=ot_ps[h][64:65, :qn],
                        scalar1=CADD, scalar2=None, op0=mybir.AluOpType.add)
                # sync queue (HWDGE, ~0.6us/issue; gpsimd SWDGE is ~1us);
                # chain hops are emitted before this chunk's y DMAs, and y
                # rides the scalar queue anyway
                zq = z_pool.tile([128, 16], f32, tag="zq", name="zq")
                nc.sync.dma_start(zq[:qp, :], zall[0:1, :, :qz])
                nc.vector.reciprocal(zq[:qp, :], zq[:qp, :])
                zd = zscr.tile([4, 512], f32, tag="zd", name="zd")
                nc.sync.dma_start(zd[0:4, :qz], zq[:qp, :])
                for h in range(HC):
                    r0 = (h % 2) * 64
                    dqb = nc.sync if h % 2 == 0 else nc.scalar
                    dqb.dma_start(rb[r0:r0 + 64, h // 2, :qn],
                                  zd[h:h + 1, :qn].to_broadcast((64, qn)))
            return fn

        def mk_stt(ot_ps, rb, osball, p, h, qn):
            def fn():
                r0 = (h % 2) * 64
                nc.vector.scalar_tensor_tensor(
                    out=osball[r0:r0 + 64, p, :qn],
                    in0=ot_ps[h][0:64, :qn],
                    scalar=1.0,
                    in1=rb[r0:r0 + 64, p, :qn],
                    op0=mybir.AluOpType.mult,
                    op1=mybir.AluOpType.mult)
            return fn

        for ci, (q0, qn) in enumerate(qord0):
            # all 4 heads at once: score matmuls rotate PE row-tile
            # positions 0/32/64/96, which is what lets LDWEIGHTS pipeline
            # (measured 62 ns/score at 4-deep rotation vs 425 ns blocked)
            tail = nst * qn <= 512
            pt = {}
            for h in range(HC):
                pt[h] = pt_pool.tile([128, nst, 512], fp8, tag="pt",
                                     name="pt")
            if not tail:
                osball = osb_pool.tile([128, 2, 512], fp8, tag="osb",
                                       name="osball")
                y_osb[(q0, qn)] = osball
                ot_ps = {}
                for h in range(HC):
                    ot_ps[h] = otp.tile([68, 512], f32, tag="ot",
                                        name="otps")
                rb = z_pool.tile([128, 2, 512], f32, tag="rb", name="rb")
            if tail:
                # everything rides the fast-recycling "st" ring: no OT-ring
                # WAR on the previous chunk's chain, no on-device r at all
                run_due()
                for h in range(HC):
                    ensure_q(h // 2, q0)
                    ensure_km(h // 2, nst - 1)
                drain(len(aux))
                for s in range(nst):
                    ensure_v(s)
                zrows = z_pool.tile([1, 4, 64], f32, tag="ztl", name="zrows")
                otu = z_pool.tile([128, 512], fp8, tag="otu", name="otu")
                for h in range(HC):
                    st = stp.tile([128, nst, qn], f32, tag="st", name="st")
                    for s in range(nst):
                        nc.tensor.matmul(
                            st[:, s, :qn],
                            kt[32 * h:32 * h + 32, :,
                               s * 128:(s + 1) * 128],
                            qt[32 * h:32 * h + 32, :, q0:q0 + qn],
                            start=True, stop=True, perf_mode=DR,
                            tile_position=(32 * h, 0))
                    nc.scalar.activation(
                        pt[h][:, :, :qn], st[:, :, :qn],
                        mybir.ActivationFunctionType.Exp,
                        scale=ESCALE, bias=ebias[:, :])
                    tick()
                for h in range(HC):
                    r0 = (h % 2) * 64
                    otps = stp.tile([128, 512], f32, tag="st", name="otps")
                    for sp in range(0, nst - 1, 2):
                        nc.tensor.matmul(
                            otps[0:68, :qn],
                            v_aug[:, sp:sp + 2, h, :],
                            pt[h][:, sp:sp + 2, :qn],
                            start=(sp == 0), stop=(sp + 2 == nst),
                            perf_mode=DR)
                    if nst % 2 == 1:
                        nc.tensor.matmul(
                            otps[0:68, :qn],
                            v_aug[:, nst - 1, h, 0:68],
                            pt[h][:, nst - 1, :qn],
                            start=(nst == 1), stop=True)
                    nc.vector.tensor_copy(zrows[0:1, h, :qn],
                                          otps[64:65, :qn])
                    # unnormalized OT can exceed fp8 range: store OT/64
                    nc.vector.tensor_scalar(
                        out=otu[r0:r0 + 64, :qn], in0=otps[0:64, :qn],
                        scalar1=1.0 / 64.0, scalar2=None,
                        op0=mybir.AluOpType.mult)
                    for di, d0 in enumerate((0, 512)):
                        yh = stp.tile([128, 512], f32, tag="st", name="yh")
                        nc.tensor.matmul(
                            yh[:qn, :512],
                            otu[r0:r0 + 64, :qn],
                            wo_sb[r0:r0 + 64, h // 2, d0:d0 + 512],
                            start=True, stop=True)
                        yho = yo_pool.tile([128, 512], bf16, tag="yo",
                                           name="yho")
                        nc.scalar.copy(yho[:qn, :], yh[:qn, :512])
                        nc.scalar.dma_start(
                            yt_d[h, 0:qn, d0:d0 + 512], yho[:qn, :])
                nc.sync.dma_start(zt_d[0:1, :, :], zrows[0:1, :, :])
                run_due(flush=True)
                continue
            else:
                for s in range(nst):
                    run_due()
                    drain(1)
                    sts = {}
                    for h in range(HC):
                        if h % 2 == 0:
                            ensure_q(h // 2, q0)
                            ensure_km(h // 2, s)
                        sts[h] = stp.tile([128, 512], f32, tag="st",
                                          name="st")
                        nc.tensor.matmul(
                            sts[h][:, :qn],
                            kt[32 * h:32 * h + 32, :,
                               s * 128:(s + 1) * 128],
                            qt[32 * h:32 * h + 32, :, q0:q0 + qn],
                            start=True, stop=True, perf_mode=DR,
                            tile_position=(32 * h, 0))
                    for h in range(HC):
                        nc.scalar.activation(
                            pt[h][:, s, :qn], sts[h][:, :qn],
                            mybir.ActivationFunctionType.Exp,
                            scale=ESCALE, bias=ebias[:, :])
                    if s % 2 == 1:
                        after(2, mk_ot_pair(ot_ps, pt, range(HC), s - 1, qn,
                                            s + 1 == nst))
                    tick()
            # emission-order constraints (in-order queues + pool-slot WAR
            # deps bind at emission): single-OT < chain < STT, and STT(c)
            # before chunk c+1's first OT; all Y(c) before STT(c+2)
            if tail:
                for j in range(0, nst - 1, 2):
                    after(1, mk_ot_pair(ot_ps, pt, range(HC), j, qn,
                                        j + 2 == nst))
            if nst % 2 == 1:
                after(1, mk_ot_single(ot_ps, pt, range(HC), nst - 1, qn))
            after(1, mk_chain_all(ot_ps, rb, qn))
            for h in range(HC):
                after(2, mk_stt(ot_ps, rb, osball, h // 2, h, qn))
            last_chunk = ci == len(qord0) - 1
            for jt in range((qn + 127) // 128):
                for hi, d0 in enumerate((0, 512)):
                    after(4 + jt + hi,
                          lambda a=q0, b=qn, j=jt, d=d0, L=last_chunk:
                          y_job(a, b, j, d, L))
        drain(len(aux))
        while vq:
            proj_v(vq.pop(0))
        run_due(flush=True)
    nc.compile()
    return nc


_nc_cache: dict = {}


def _get_nc(SAQ, SAK):
    key = (SAQ, SAK)
    if key not in _nc_cache:
        _nc_cache[key] = _build(SAQ, SAK)
    return _nc_cache[key]


def _score_perm():
    """W column permutation (within each head-group's 256 cols) for the
    DR-32 layout: psum partitions [0:64] of the m-half are the j=0 subtile
    (dims 0..31 of heads 2m, 2m+1), [64:128] the j=1 subtile (dims 32..63),
    so head h sits at qt[32h:32h+32, j, :] with dim d = j*32 + p%32."""
    perm = np.empty(2 * 128, np.int64)
    for m in range(2):
        for p in range(128):
            h = 2 * m + (p % 64) // 32
            d = (p // 64) * 32 + (p % 32)
            perm[m * 128 + p] = h * 64 + d
    return perm


def _shuf(a):
    """[D, C] -> [128, D//128, C] host-contiguous fp8 (partition-major)."""
    Dd, C = a.shape
    return np.ascontiguousarray(
        a.reshape(Dd // 128, 128, C).transpose(1, 0, 2)).astype(npf8)


def _reference_fallback(x, gate, Wq, bq, Wk, bk, Wv, bv, Wo, bo):
    g = gate.astype(x.dtype)[..., None]
    q = (x @ Wq + bq) * g
    k = (x @ Wk + bk) * g
    v = (x @ Wv + bv) * g

    def split(t):
        return t.reshape(B, S, H, DH).transpose(0, 2, 1, 3)

    q, k, v = split(q), split(k), split(v)
    sc = np.einsum('bhqd,bhkd->bhqk', q, k) / np.float32(math.sqrt(DH))
    sc = sc - sc.max(axis=-1, keepdims=True)
    e = np.exp(sc)
    attn = e / e.sum(axis=-1, keepdims=True)
    out = np.einsum('bhqk,bhkd->bhqd', attn, v)
    out = out.transpose(0, 2, 1, 3).reshape(B, S, D)
    out = out @ Wo + bo
    return (x * (1.0 - g) + out * g).astype(np.float32)


def kernel(x, gate, Wq, bq, Wk, bk, Wv, bv, Wo, bo, _profile=None):
    x = np.asarray(x, np.float32)
    gate = np.asarray(gate)
    args = dict(x=x, gate=gate, Wq=np.asarray(Wq, np.float32),
                bq=np.asarray(bq, np.float32), Wk=np.asarray(Wk, np.float32),
                bk=np.asarray(bk, np.float32), Wv=np.asarray(Wv, np.float32),
                bv=np.asarray(bv, np.float32), Wo=np.asarray(Wo, np.float32),
                bo=np.asarray(bo, np.float32))

    idxs = [np.nonzero(gate[b])[0] for b in range(B)]
    n_act = [len(i) for i in idxs]
    # the compaction trick needs zero q/k/v biases and at least one active
    # and one inactive token per batch; otherwise fall back to exact numpy
    if (any(np.abs(args[k]).max() > 0 for k in ("bq", "bk", "bv"))
            or min(n_act) == 0 or max(n_act) == S):
        return _reference_fallback(**args)

    SAQ = max(n_act)
    SAK = ((SAQ + 127) // 128) * 128
    perm = _score_perm()

    in_maps = []
    for b in range(B):
        xa = np.zeros((SAK, D), np.float32)
        xa[:n_act[b]] = x[b, idxs[b]]
        xT = _shuf(np.ascontiguousarray(xa.T))
        for g in range(GROUPS):
            cs = slice(g * DC, (g + 1) * DC)
            in_maps.append({
                "xT": xT,
                "wq": _shuf(args["Wq"][:, cs][:, perm] * WSCALE),
                "wk": _shuf(args["Wk"][:, cs][:, perm] * WSCALE),
                "wv": _shuf(args["Wv"][:, cs] * WSCALE),
                "wo": np.ascontiguousarray(
                    (args["Wo"][cs, :] * WSCALE).reshape(2, 128, D)
                    .transpose(1, 0, 2)).astype(npf8),
            })

    nc = _get_nc(SAQ, SAK)
    kw = dict(_profile) if _profile else {}
    kw.pop("result", None)
    res = run_bass_kernel_spmd(nc, in_maps, core_ids=list(range(NCORES)), **kw)
    if _profile is not None:
        _profile["result"] = res

    out = x.copy()
    inv = 1.0 / (WSCALE * WSCALE)
    nst = SAK // 128
    qch = []
    o = 0
    while o < SAQ:
        qch.append((o, min(512, SAQ - o)))
        o += 512
    tq0, tqn = qch[-1]
    has_tail = nst * tqn <= 512
    cadd = float(S - SAK) / 16.0
    for b in range(B):
        Y = np.zeros((SAQ, D), np.float32)
        for g in range(GROUPS):
            r = res.results[b * GROUPS + g]
            yv = r["y"].astype(np.float32)
            if has_tail:
                yv[tq0:] = 0.0   # device never writes tail rows of y
            Y += yv
            if has_tail:
                # narrow-tail rows: per-head unnormalized Y + raw Z; apply
                # the softmax denominator here
                yt = np.asarray(r["yt"], np.float32)    # [4, 64, D]
                zt = np.asarray(r["zt"], np.float32)[0]  # [4, 64]
                rr = 64.0 / (zt[:, :tqn] + cadd)         # [4, tqn]
                Y[tq0:tq0 + tqn] += np.einsum(
                    'hqd,hq->qd', yt[:, :tqn], rr)
        out[b, idxs[b]] = Y[:n_act[b]] * inv + args["bo"]
    return out


# revision 43
# speedup vs baseline: 1.2129x; 1.0205x over previous
"""Entropy-gated multi-head attention on 8 Trainium2 NeuronCores.

Sharding: core c = b*4 + g handles batch b (of 2) and head-group g (4 of the
16 heads).  Tokens with gate==0 pass x through untouched and contribute
exactly zero k/v (zero biases), so the device only processes the compacted
active tokens (~half), with the softmax denominator corrected by the count of
inactive tokens: each inactive key contributes exp(0)=1 to the softmax sum
(scores vs. zeroed k are exactly 0) and nothing to the numerator (v=0).

Device math per core (no max-subtraction; scores are O(5) so exp is safe):
  QT = Wq_g^T x^T, KT = Wk_g^T x^T           [256, SA]
  V  = x Wv_g                                 [SA, 256]
  per head h: PT = exp((KT_h^T QT_h)/8)/16    [SA_k, SA_q]  (fp8, DoubleRow)
              OT' = [V_h | 1]^T PT            [65, SA_q] (row 64 = colsum)
              r = 1/(Z + (S - SA)/16)         broadcast to [64, SA_q] via DMA
              osb_h = OT * r                  (scaled attention out, lhsT form)
  Y(q, :) = sum_h osb_h^T Wo_h                (K=128 pair-packed psum accum)
Host sums the 4 per-group partial Y per batch, adds bo, scatters into x.

fp8 path: scores contract DH=64 as [32 partitions x 2 k-subtiles] DoubleRow
(W columns host-permuted so QT/KT psum halves land directly in the
[128, 2, SA] layout); OT contracts key-tile pairs as [128, 2, 65] DoubleRow
with PT written by ACT as exp(s/8)/16 in fp8e4 (1/16 keeps exp under the
+-240 fp8e4 clip; the softmax correction uses CADD/16 to compensate).
"""

import math
from contextlib import ExitStack

import numpy as np
import ml_dtypes

import concourse.bass as bass
import concourse.mybir as mybir
from concourse import bacc
import concourse.tile as tile
from concourse.bass_utils import run_bass_kernel_spmd

B, S, D = 2, 2048, 1024
H, DH = 16, 64
NCORES = 8
GROUPS = NCORES // B          # head-groups per batch = 4
HC = H // GROUPS              # heads per core = 4
DC = HC * DH                  # head-group width = 256

# matmul operand dtype for projections/Y: "bf16" | "f32" | "f32r"
MM_DTYPE = "bf16"
SCORE_FP8 = False             # 32-row DR scores stream slower than bf16: keep bf16
OT_FP8 = True                 # fp8e4 pt/v_aug, DoubleRow over key-tile pairs
PROJ_FP8 = True               # fp8e4 x/Wq/Wk/Wv, DoubleRow over k-tile pairs
Y_FP8 = False                 # DR Y saved no wall time (fill-in work); bf16
# W host-scale 16 keeps fp8 weights in the normal range; compensated by
# exp-scale/256 (q,k both x16) and the osb output scale 1/16 (v x16)
WSCALE = 16.0

_DT = {
    "bf16": mybir.dt.bfloat16,
    "f32": mybir.dt.float32,
    "f32r": mybir.dt.float32,
}
_NPDT = {
    "bf16": ml_dtypes.bfloat16,
    "f32": np.float32,
    "f32r": np.float32,
}

f32 = mybir.dt.float32
fp8 = mybir.dt.float8e4
LN16 = float(np.log(16.0))
DR = mybir.MatmulPerfMode.DoubleRow


def _chunks(total, step):
    out = []
    o = 0
    while o < total:
        out.append((o, min(step, total - o)))
        o += step
    return out


def _build(SA: int, dtype_tag: str) -> bass.Bass:
    DT = _DT[dtype_tag]
    SDT = fp8 if SCORE_FP8 else DT   # qt/kt storage
    PDT = fp8 if OT_FP8 else DT      # pt/v_aug storage
    XDT = fp8 if PROJ_FP8 else DT    # x / Wq / Wk / Wv storage
    PT_SCALE = 16.0 if OT_FP8 else 1.0
    if PROJ_FP8:
        assert OT_FP8, "PROJ_FP8 requires the fp8 OT path (scale bookkeeping)"
    # with PROJ_FP8, q/k carry a WSCALE factor each -> scores are WSCALE^2 up
    ESCALE = 0.125 / (WSCALE * WSCALE if PROJ_FP8 else 1.0)

    def mm(ap):
        if dtype_tag == "f32r":
            return ap.bitcast(mybir.dt.float32r)
        return ap

    nkt = D // 128            # 8 contraction tiles for projections
    nst = SA // 128           # token tiles
    qch = _chunks(SA, 512)    # q chunks
    dch = _chunks(D, 512)     # output-dim chunks
    CADD = float(S - SA) / PT_SCALE

    nc = bacc.Bacc()
    xT_d = nc.dram_tensor("xT", [D, SA], XDT, kind="ExternalInput")
    wq_d = nc.dram_tensor("wq", [D, DC], XDT, kind="ExternalInput")
    wk_d = nc.dram_tensor("wk", [D, DC], XDT, kind="ExternalInput")
    wv_d = nc.dram_tensor("wv", [D, DC], XDT, kind="ExternalInput")
    WODT = fp8 if Y_FP8 else DT
    wo_d = nc.dram_tensor("wo", [DC, D], WODT, kind="ExternalInput")
    y_d = nc.dram_tensor("y", [SA, D], f32, kind="ExternalOutput")

    with tile.TileContext(nc) as tc, ExitStack() as ctx:
        singles = ctx.enter_context(tc.tile_pool(name="singles", bufs=1))
        pt_pool = ctx.enter_context(tc.tile_pool(name="pt", bufs=6))
        otsb_pool = ctx.enter_context(tc.tile_pool(name="otsb", bufs=2))
        zr_pool = ctx.enter_context(tc.tile_pool(name="zr", bufs=2))
        zq_pool = ctx.enter_context(tc.tile_pool(name="zq", bufs=2))
        rbc_pool = ctx.enter_context(tc.tile_pool(name="rbc", bufs=2))
        yout_pool = ctx.enter_context(tc.tile_pool(name="yout", bufs=3))
        zscr_pool = ctx.enter_context(tc.tile_pool(name="zscr", bufs=8,
                                                   space="DRAM"))
        # PSUM: st pairs 2x2 banks + aux (proj/V/Y) 2x1 + ot 2x1 = 8 banks
        mm_ps = ctx.enter_context(tc.tile_pool(name="mmps", bufs=2, space="PSUM"))
        aux_ps = ctx.enter_context(tc.tile_pool(name="auxps", bufs=2, space="PSUM"))
        ot_ps_pool = ctx.enter_context(tc.tile_pool(name="otps", bufs=2, space="PSUM"))

        # ---- persistent SBUF; one batched DMA per tensor/chunk keeps the
        # sync-queue issue time (~0.6us per dma_start) off the startup path
        wq_sb = singles.tile([128, nkt, DC], XDT)
        wk_sb = singles.tile([128, nkt, DC], XDT)
        wv_sb = singles.tile([128, nkt, DC], XDT)
        xt = singles.tile([128, nkt, SA], XDT)
        (c0, c0n) = qch[0]
        nc.sync.dma_start(wq_sb[:, :, :],
                          wq_d[:, :].rearrange("(t p) c -> p t c", p=128))
        nc.sync.dma_start(xt[:, :, c0:c0 + c0n],
                          xT_d[:, c0:c0 + c0n].rearrange(
                              "(t p) q -> p t q", p=128))
        nc.sync.dma_start(wk_sb[:, :, :],
                          wk_d[:, :].rearrange("(t p) c -> p t c", p=128))
        for (q0, qn) in qch[1:]:
            nc.sync.dma_start(xt[:, :, q0:q0 + qn],
                              xT_d[:, q0:q0 + qn].rearrange(
                                  "(t p) q -> p t q", p=128))
        nc.sync.dma_start(wv_sb[:, :, :],
                          wv_d[:, :].rearrange("(t p) c -> p t c", p=128))
        if Y_FP8:
            # [128, 2, D]: pair p rows as DR subtile p
            wo2 = singles.tile([128, 2, D], WODT, tag="wo2", name="wo2")
            nc.sync.dma_start(wo2[:, :, :],
                              wo_d[:, :].rearrange("(p k) d -> k p d", p=2))
            wo_sb = []
        else:
            wo_sb = []
            for p in range(HC // 2):
                w = singles.tile([128, D], DT, tag=f"wo{p}", name=f"wo{p}")
                nc.sync.dma_start(w, wo_d[p * 128:(p + 1) * 128, :])
                wo_sb.append(w)

        # ---- projections ----
        if SCORE_FP8:
            # [128, 2, SA]: head h on partitions 32h..32h+32, dim d of head h
            # at (p = 32h + d%32, j = d//32); W cols are host-permuted so the
            # m-th psum half is exactly the j=m slice.
            qt = singles.tile([128, 2, SA], SDT, tag="qt", name="qt")
            kt = singles.tile([128, 2, SA], SDT, tag="kt", name="kt")
        else:
            qt = [singles.tile([128, SA], DT, tag=f"qt{m}", name=f"qt{m}")
                  for m in range(2)]
            kt = [singles.tile([128, SA], DT, tag=f"kt{m}", name=f"kt{m}")
                  for m in range(2)]
        # 68 cols (V | ones | zero-pad): the dual-fp8 LDWEIGHTS subtile
        # stride (HC*cols) must be a multiple of 16, so cols % 4 == 0
        v_aug = singles.tile([128, nst, HC, 68], PDT)

        def proj_qk(m, dst, w_sb, q0, qn):
            ps = aux_ps.tile([128, 512], f32, tag="aux", name="ps")
            if PROJ_FP8:
                for t2 in range(nkt // 2):
                    nc.tensor.matmul(
                        ps[:, :qn],
                        w_sb[:, 2 * t2:2 * t2 + 2, m * 128:(m + 1) * 128],
                        xt[:, 2 * t2:2 * t2 + 2, q0:q0 + qn],
                        start=(t2 == 0), stop=(t2 == nkt // 2 - 1),
                        perf_mode=DR)
            else:
                for t in range(nkt):
                    nc.tensor.matmul(
                        ps[:, :qn],
                        mm(w_sb[:, t, m * 128:(m + 1) * 128]),
                        mm(xt[:, t, q0:q0 + qn]),
                        start=(t == 0), stop=(t == nkt - 1))
            if SCORE_FP8:
                # m-half m = heads (2m, 2m+1): psum [0:64] is their j=0
                # subtile, [64:128] their j=1 (cross-partition-base copy)
                nc.vector.tensor_copy(dst[64 * m:64 * m + 64, 0, q0:q0 + qn],
                                      ps[0:64, :qn])
                nc.vector.tensor_copy(dst[64 * m:64 * m + 64, 1, q0:q0 + qn],
                                      ps[64:128, :qn])
            else:
                nc.vector.tensor_copy(dst[m][:, q0:q0 + qn], ps[:, :qn])

        v_done = set()

        def proj_v(s):
            v_done.add(s)
            ps = aux_ps.tile([128, 512], f32, tag="aux", name="ps")
            if PROJ_FP8:
                for t2 in range(nkt // 2):
                    nc.tensor.matmul(
                        ps[:, :DC],
                        xt[:, 2 * t2:2 * t2 + 2, s * 128:(s + 1) * 128],
                        wv_sb[:, 2 * t2:2 * t2 + 2, :],
                        start=(t2 == 0), stop=(t2 == nkt // 2 - 1),
                        perf_mode=DR)
            else:
                for t in range(nkt):
                    nc.tensor.matmul(
                        ps[:, :DC],
                        mm(xt[:, t, s * 128:(s + 1) * 128]),
                        mm(wv_sb[:, t, :]),
                        start=(t == 0), stop=(t == nkt - 1))
            for h in range(HC):
                nc.vector.tensor_copy(v_aug[:, s, h, 0:64],
                                      ps[:, h * 64:(h + 1) * 64])

        # m=0 projections and the first V tiles run up-front (PE-dense, warms
        # HAM); remaining independent PE work (V tail, m=1 projections, Y of
        # finished chunks) is queued and drained between attention groups so
        # the PE never starves while ACT runs the exps.
        aux_jobs = []

        def drain_aux(k):
            for _ in range(min(k, len(aux_jobs))):
                aux_jobs.pop(0)()

        nc.vector.memset(v_aug[:, :, :, 64:65], 1.0)
        nc.vector.memset(v_aug[:, :, :, 65:68], 0.0)
        ebias = None
        if OT_FP8:
            # exp bias -ln16: pt = exp(s/8)/16 keeps exp under the fp8e4 clip
            ebias = singles.tile([128, 1], f32, tag="ebias", name="ebias")
            nc.vector.memset(ebias, -LN16)
        for (q0, qn) in qch:
            proj_qk(0, qt, wq_sb, q0, qn)
            proj_qk(0, kt, wk_sb, q0, qn)
        proj_v(0)
        proj_v(1)
        for s in range(2, nst):
            aux_jobs.append(lambda s=s: proj_v(s))
        for (q0, qn) in qch:
            aux_jobs.append(lambda a=q0, b=qn: proj_qk(1, qt, wq_sb, a, b))
            aux_jobs.append(lambda a=q0, b=qn: proj_qk(1, kt, wk_sb, a, b))

        # ---- attention + output projection, per q chunk ----
        for ci, (q0, qn) in enumerate(qch):
            ot_sb = [None] * HC
            osball = (otsb_pool.tile([128, 2, 512], fp8, tag="osball",
                                     name="osball") if Y_FP8 else None)
            for p in range(HC // 2):
                m = p
                if p == 1 and ci == 0:
                    # pair 1 needs the m=1 projections: pull them forward
                    drain_aux(len(aux_jobs))
                ot_ps = {}
                for h in (2 * p, 2 * p + 1):
                    ot_ps[h] = ot_ps_pool.tile([68, 512], f32, tag="ot",
                                               name="ot_ps")
                for si in range(0, nst, 2):
                    # NOTE: emission order is semantic order in Tile — a
                    # consumer emitted before its producer reads stale data.
                    # During chunk0/pair0 the V-projection jobs at the queue
                    # head MUST outpace the OT consumers: 2 jobs per si-group
                    # keeps proj_v(s) strictly ahead of OT reads of v_aug[s].
                    drain_aux(2 if (ci == 0 and p == 0) else 1)
                    npair = min(2, nst - si)
                    assert all(s in v_done for s in range(si, si + npair)), \
                        f"proj_v not emitted before OT consumer: {si}"

                    st, pt = {}, {}
                    for h in (2 * p, 2 * p + 1):
                        st[h] = mm_ps.tile([128, 2, 512], f32, tag="mm",
                                           name="st_ps")
                    for j in range(npair):
                        s = si + j
                        for h in (2 * p, 2 * p + 1):
                            if SCORE_FP8:
                                nc.tensor.matmul(
                                    st[h][:, j, :qn],
                                    kt[32 * h:32 * h + 32, :,
                                       s * 128:(s + 1) * 128],
                                    qt[32 * h:32 * h + 32, :, q0:q0 + qn],
                                    start=True, stop=True,
                                    perf_mode=DR,
                                    tile_position=(32 * h, 0))
                            else:
                                # adjacent (even,odd) matmuls with explicit
                                # row-group tile_position pack the PE array
                                r0 = (h % 2) * 64
                                nc.tensor.matmul(
                                    st[h][:, j, :qn],
                                    mm(kt[m][r0:r0 + 64,
                                             s * 128:(s + 1) * 128]),
                                    mm(qt[m][r0:r0 + 64, q0:q0 + qn]),
                                    start=True, stop=True,
                                    tile_position=(r0, 0))
                    for h in (2 * p, 2 * p + 1):
                        pt[h] = pt_pool.tile([128, 2, 512], PDT, tag="pt",
                                             name="pt")
                        nc.scalar.activation(
                            pt[h][:, :npair, :qn], st[h][:, :npair, :qn],
                            mybir.ActivationFunctionType.Exp,
                            scale=ESCALE,
                            bias=(ebias[:, :] if OT_FP8 else 0.0))
                    for h in (2 * p, 2 * p + 1):
                        if OT_FP8 and npair == 2:
                            nc.tensor.matmul(
                                ot_ps[h][:, :qn],
                                v_aug[:, si:si + 2, h, :],
                                pt[h][:, 0:2, :qn],
                                start=(si == 0), stop=(si + 1 == nst - 1),
                                perf_mode=DR)
                        else:
                            for j in range(npair):
                                s = si + j
                                nc.tensor.matmul(
                                    ot_ps[h][:, :qn],
                                    mm(v_aug[:, s, h, 0:68]) if not OT_FP8
                                    else v_aug[:, s, h, 0:68],
                                    mm(pt[h][:, j, :qn]) if not OT_FP8
                                    else pt[h][:, j, :qn],
                                    start=(s == 0), stop=(s == nst - 1))
                if Y_FP8:
                    osbp = osball[:, p, :]
                else:
                    osbp = otsb_pool.tile([128, 512], DT, tag=f"osbp{p}",
                                          name=f"osbp{p}")
                ot_sb[p] = osbp
                for h in (2 * p, 2 * p + 1):
                    # evacuate psum immediately (zt row + unscaled OT copy,
                    # both cheap DVE) so the bank frees for the next chunk's
                    # OT; the r chain (gpsimd-queue DMA hops) then runs off
                    # the critical path; osb is scaled before the Y matmuls.
                    r0 = 0 if h % 2 == 0 else 64
                    zt = zr_pool.tile([65, 512], f32, tag="zt", name="zt")
                    nc.vector.tensor_scalar(
                        out=zt[64:65, :qn], in0=ot_ps[h][64:65, :qn],
                        scalar1=CADD, scalar2=None, op0=mybir.AluOpType.add)
                    otu = otsb_pool.tile([128, 512], DT,
                                         tag=f"otu{h - 2 * p}", name="otu")
                    nc.vector.tensor_copy(otu[r0:r0 + 64, :qn],
                                          ot_ps[h][0:64, :qn])
                    # z [1,qn] -> [128,4] so the reciprocal runs wide on DVE;
                    # alternate issue queue by head parity so the two chains
                    # of a pair don't serialize on one sequencer at endgame
                    dq = nc.gpsimd if h % 2 == 0 else nc.sync
                    zq = zq_pool.tile([128, 4], f32, tag="zq", name="zq")
                    qp = (qn + 3) // 4
                    dq.dma_start(zq[:qp, :], zt[64:65, :qn])
                    nc.vector.reciprocal(zq[:qp, :], zq[:qp, :])
                    zd2 = zscr_pool.tile([1, 512], f32, tag="zd2", name="zd2")
                    dq.dma_start(zd2[0:1, :qn], zq[:qp, :])
                    rb = rbc_pool.tile([128, 512], f32, tag=f"rbc{h}",
                                       name=f"rbc{h}")
                    dq.dma_start(rb[r0:r0 + 64, :qn],
                                 zd2[0:1, :qn].to_broadcast((64, qn)))
                    nc.vector.scalar_tensor_tensor(
                        out=osbp[r0:r0 + 64, :qn],
                        in0=otu[r0:r0 + 64, :qn],
                        scalar=(1.0 / WSCALE if PROJ_FP8 else 1.0),
                        in1=rb[r0:r0 + 64, :qn],
                        op0=mybir.AluOpType.mult,
                        op1=mybir.AluOpType.mult)

            # output projection: osbp packs the head pair on 128 partitions,
            # so each pair is a single K=128 accumulating matmul per output
            # chunk.  p-outer/dch-inner keeps the stationary operand
            # back-to-back for weight-reuse.  Queued so the Y matmuls fill
            # PE bubbles of the next chunk's (ACT-bound) attention.
            def y_job(q0, qn, jt, osb_pair, osball=None):
                qtn = min(128, qn - jt * 128)
                yps = [aux_ps.tile([128, 512], f32, tag="aux", name=f"y{di}")
                       for di in range(len(dch))]
                for di, (d0, dn) in enumerate(dch):
                    if Y_FP8:
                        nc.tensor.matmul(
                            yps[di][:qtn, :dn],
                            osball[:, :, jt * 128:jt * 128 + qtn],
                            wo2[:, :, d0:d0 + dn],
                            start=True, stop=True, perf_mode=DR)
                    else:
                        for p in range(HC // 2):
                            nc.tensor.matmul(
                                yps[di][:qtn, :dn],
                                mm(osb_pair[p][:, jt * 128:jt * 128 + qtn]),
                                mm(wo_sb[p][:, d0:d0 + dn]),
                                start=(p == 0), stop=(p == HC // 2 - 1))
                for di, (d0, dn) in enumerate(dch):
                    yo = yout_pool.tile([128, 512], f32, tag="yo", name="yo")
                    if Y_FP8:
                        nc.vector.tensor_scalar(
                            out=yo[:qtn, :dn], in0=yps[di][:qtn, :dn],
                            scalar1=1.0 / WSCALE, scalar2=None,
                            op0=mybir.AluOpType.mult)
                    else:
                        nc.vector.tensor_copy(yo[:qtn, :dn], yps[di][:qtn, :dn])
                    # scalar queue: the sync queue is congested with
                    # end-of-kernel semaphores right when the last y lands
                    nc.scalar.dma_start(
                        y_d[q0 + jt * 128: q0 + jt * 128 + qtn, d0:d0 + dn],
                        yo[:qtn, :dn])

            for jt in range((qn + 127) // 128):
                aux_jobs.append(
                    lambda a=q0, b=qn, j=jt, o=tuple(ot_sb[:HC // 2]),
                    ob=osball: y_job(a, b, j, o, ob))
        drain_aux(len(aux_jobs))
    nc.compile()
    return nc


_nc_cache: dict = {}


def _get_nc(SA: int):
    key = (SA, MM_DTYPE, SCORE_FP8, OT_FP8, PROJ_FP8, Y_FP8)
    if key not in _nc_cache:
        _nc_cache[key] = _build(SA, MM_DTYPE)
    return _nc_cache[key]


def _score_perm():
    """W column permutation (within each head-group's 256 cols): m-half m
    covers heads (2m, 2m+1); psum partitions [0:64] are their j=0 subtile
    (dims 0..31), [64:128] the j=1 subtile (dims 32..63), so head h sits at
    qt[32h:32h+32, j, :] with dim d = j*32 + p%32."""
    perm = np.empty(2 * 128, np.int64)
    for m in range(2):
        for p in range(128):
            h = 2 * m + (p % 64) // 32
            d = (p // 64) * 32 + (p % 32)
            perm[m * 128 + p] = h * 64 + d
    return perm


def _reference_fallback(x, gate, Wq, bq, Wk, bk, Wv, bv, Wo, bo):
    g = gate.astype(x.dtype)[..., None]
    q = (x @ Wq + bq) * g
    k = (x @ Wk + bk) * g
    v = (x @ Wv + bv) * g

    def split(t):
        return t.reshape(B, S, H, DH).transpose(0, 2, 1, 3)

    q, k, v = split(q), split(k), split(v)
    sc = np.einsum('bhqd,bhkd->bhqk', q, k) / np.float32(math.sqrt(DH))
    sc = sc - sc.max(axis=-1, keepdims=True)
    e = np.exp(sc)
    attn = e / e.sum(axis=-1, keepdims=True)
    out = np.einsum('bhqk,bhkd->bhqd', attn, v)
    out = out.transpose(0, 2, 1, 3).reshape(B, S, D)
    out = out @ Wo + bo
    return (x * (1.0 - g) + out * g).astype(np.float32)


def kernel(x, gate, Wq, bq, Wk, bk, Wv, bv, Wo, bo, _profile=None):
    x = np.asarray(x, np.float32)
    gate = np.asarray(gate)
    args = dict(x=x, gate=gate, Wq=np.asarray(Wq, np.float32),
                bq=np.asarray(bq, np.float32), Wk=np.asarray(Wk, np.float32),
                bk=np.asarray(bk, np.float32), Wv=np.asarray(Wv, np.float32),
                bv=np.asarray(bv, np.float32), Wo=np.asarray(Wo, np.float32),
                bo=np.asarray(bo, np.float32))

    idxs = [np.nonzero(gate[b])[0] for b in range(B)]
    n_act = [len(i) for i in idxs]
    # the compaction trick needs zero q/k/v biases and at least one active
    # and one inactive token per batch; otherwise fall back to exact numpy
    if (any(np.abs(args[k]).max() > 0 for k in ("bq", "bk", "bv"))
            or min(n_act) == 0 or max(n_act) == S):
        return _reference_fallback(**args)

    SA = ((max(n_act) + 127) // 128) * 128
    npdt = _NPDT[MM_DTYPE]
    xdt = ml_dtypes.float8_e4m3 if PROJ_FP8 else npdt
    ws = WSCALE if PROJ_FP8 else 1.0
    perm = _score_perm() if SCORE_FP8 else np.arange(256)

    in_maps = []
    for b in range(B):
        xa = np.zeros((SA, D), np.float32)
        xa[:n_act[b]] = x[b, idxs[b]]
        xT = np.ascontiguousarray(xa.T).astype(xdt)
        for g in range(GROUPS):
            cs = slice(g * DC, (g + 1) * DC)
            in_maps.append({
                "xT": xT,
                "wq": np.ascontiguousarray(
                    args["Wq"][:, cs][:, perm] * ws).astype(xdt),
                "wk": np.ascontiguousarray(
                    args["Wk"][:, cs][:, perm] * ws).astype(xdt),
                "wv": np.ascontiguousarray(
                    args["Wv"][:, cs] * ws).astype(xdt),
                "wo": np.ascontiguousarray(
                    args["Wo"][cs, :] * (WSCALE if Y_FP8 else 1.0)).astype(
                    ml_dtypes.float8_e4m3 if Y_FP8 else npdt),
            })

    nc = _get_nc(SA)
    kw = dict(_profile) if _profile else {}
    kw.pop("result", None)
    res = run_bass_kernel_spmd(nc, in_maps, core_ids=list(range(NCORES)), **kw)
    if _profile is not None:
        _profile["result"] = res

    out = x.copy()
    for b in range(B):
        Y = np.zeros((SA, D), np.float32)
        for g in range(GROUPS):
            Y += res.results[b * GROUPS + g]["y"]
        out[b, idxs[b]] = Y[:n_act[b]] + args["bo"]
    return out



# revision 45
# speedup vs baseline: 1.2647x; 1.0427x over previous
"""Entropy-gated multi-head attention on 8 Trainium2 NeuronCores.

Sharding: core c = b*4 + g handles batch b (of 2) and head-group g (4 of the
16 heads).  Tokens with gate==0 pass x through untouched and contribute
exactly zero k/v (zero biases), so the device only processes the compacted
active tokens (~half), with the softmax denominator corrected by the count of
inactive tokens: each inactive key contributes exp(0)=1 to the softmax sum
(scores vs. zeroed k are exactly 0) and nothing to the numerator (v=0).

Device math per core (no max-subtraction; scores are O(5) so exp is safe):
  QT = Wq_g^T x^T, KT = Wk_g^T x^T           [256, SA]
  V  = x Wv_g                                 [SA, 256]
  per head h: PT = exp((KT_h^T QT_h)/8)/16    [SA_k, SA_q]  (fp8, DoubleRow)
              OT' = [V_h | 1]^T PT            [65, SA_q] (row 64 = colsum)
              r = 1/(Z + (S - SA)/16)         broadcast to [64, SA_q] via DMA
              osb_h = OT * r                  (scaled attention out, lhsT form)
  Y(q, :) = sum_h osb_h^T Wo_h                (K=128 pair-packed psum accum)
Host sums the 4 per-group partial Y per batch, adds bo, scatters into x.

fp8 path: scores contract DH=64 as [32 partitions x 2 k-subtiles] DoubleRow
(W columns host-permuted so QT/KT psum halves land directly in the
[128, 2, SA] layout); OT contracts key-tile pairs as [128, 2, 65] DoubleRow
with PT written by ACT as exp(s/8)/16 in fp8e4 (1/16 keeps exp under the
+-240 fp8e4 clip; the softmax correction uses CADD/16 to compensate).
"""

import math
from contextlib import ExitStack

import numpy as np
import ml_dtypes

import concourse.bass as bass
import concourse.mybir as mybir
from concourse import bacc
import concourse.tile as tile
from concourse.bass_utils import run_bass_kernel_spmd

B, S, D = 2, 2048, 1024
H, DH = 16, 64
NCORES = 8
GROUPS = NCORES // B          # head-groups per batch = 4
HC = H // GROUPS              # heads per core = 4
DC = HC * DH                  # head-group width = 256

# matmul operand dtype for projections/Y: "bf16" | "f32" | "f32r"
MM_DTYPE = "bf16"
SCORE_FP8 = False             # 32-row DR scores stream slower than bf16: keep bf16
OT_FP8 = True                 # fp8e4 pt/v_aug, DoubleRow over key-tile pairs
PROJ_FP8 = True               # fp8e4 x/Wq/Wk/Wv, DoubleRow over k-tile pairs
Y_FP8 = False                 # DR Y saved no wall time (fill-in work); bf16
# W host-scale 16 keeps fp8 weights in the normal range; compensated by
# exp-scale/256 (q,k both x16) and the osb output scale 1/16 (v x16)
WSCALE = 16.0

_DT = {
    "bf16": mybir.dt.bfloat16,
    "f32": mybir.dt.float32,
    "f32r": mybir.dt.float32,
}
_NPDT = {
    "bf16": ml_dtypes.bfloat16,
    "f32": np.float32,
    "f32r": np.float32,
}

f32 = mybir.dt.float32
bf16 = mybir.dt.bfloat16
fp8 = mybir.dt.float8e4
LN16 = float(np.log(16.0))
DR = mybir.MatmulPerfMode.DoubleRow


def _chunks(total, step):
    out = []
    o = 0
    while o < total:
        out.append((o, min(step, total - o)))
        o += step
    return out


def _build(SA: int, dtype_tag: str) -> bass.Bass:
    DT = _DT[dtype_tag]
    SDT = fp8 if SCORE_FP8 else DT   # qt/kt storage
    PDT = fp8 if OT_FP8 else DT      # pt/v_aug storage
    XDT = fp8 if PROJ_FP8 else DT    # x / Wq / Wk / Wv storage
    PT_SCALE = 16.0 if OT_FP8 else 1.0
    if PROJ_FP8:
        assert OT_FP8, "PROJ_FP8 requires the fp8 OT path (scale bookkeeping)"
    # with PROJ_FP8, q/k carry a WSCALE factor each -> scores are WSCALE^2 up
    ESCALE = 0.125 / (WSCALE * WSCALE if PROJ_FP8 else 1.0)

    def mm(ap):
        if dtype_tag == "f32r":
            return ap.bitcast(mybir.dt.float32r)
        return ap

    nkt = D // 128            # 8 contraction tiles for projections
    nst = SA // 128           # token tiles
    qch = _chunks(SA, 512)    # q chunks
    dch = _chunks(D, 512)     # output-dim chunks
    CADD = float(S - SA) / PT_SCALE

    nc = bacc.Bacc()
    # host pre-shuffles to [128, t, cols] so every input DMA is 128 big
    # contiguous descriptors (vs ~1k strided rows via rearrange)
    xT_d = nc.dram_tensor("xT", [128, nkt, SA], XDT, kind="ExternalInput")
    wq_d = nc.dram_tensor("wq", [128, nkt, DC], XDT, kind="ExternalInput")
    wk_d = nc.dram_tensor("wk", [128, nkt, DC], XDT, kind="ExternalInput")
    wv_d = nc.dram_tensor("wv", [128, nkt, DC], XDT, kind="ExternalInput")
    WODT = fp8 if Y_FP8 else DT
    wo_d = nc.dram_tensor("wo", [DC, D], WODT, kind="ExternalInput")
    y_d = nc.dram_tensor("y", [SA, D], bf16, kind="ExternalOutput")

    with tile.TileContext(nc) as tc, ExitStack() as ctx:
        singles = ctx.enter_context(tc.tile_pool(name="singles", bufs=1))
        pt_pool = ctx.enter_context(tc.tile_pool(name="pt", bufs=6))
        otsb_pool = ctx.enter_context(tc.tile_pool(name="otsb", bufs=2))
        zr_pool = ctx.enter_context(tc.tile_pool(name="zr", bufs=2))
        zq_pool = ctx.enter_context(tc.tile_pool(name="zq", bufs=2))
        rbc_pool = ctx.enter_context(tc.tile_pool(name="rbc", bufs=2))
        yout_pool = ctx.enter_context(tc.tile_pool(name="yout", bufs=3))
        zscr_pool = ctx.enter_context(tc.tile_pool(name="zscr", bufs=8,
                                                   space="DRAM"))
        # PSUM: st pairs 2x2 banks + aux (proj/V/Y) 2x1 + ot 2x1 = 8 banks
        mm_ps = ctx.enter_context(tc.tile_pool(name="mmps", bufs=2, space="PSUM"))
        aux_ps = ctx.enter_context(tc.tile_pool(name="auxps", bufs=2, space="PSUM"))
        ot_ps_pool = ctx.enter_context(tc.tile_pool(name="otps", bufs=2, space="PSUM"))

        # ---- persistent SBUF; one batched DMA per tensor/chunk keeps the
        # sync-queue issue time (~0.6us per dma_start) off the startup path
        wq_sb = singles.tile([128, nkt, DC], XDT)
        wk_sb = singles.tile([128, nkt, DC], XDT)
        wv_sb = singles.tile([128, nkt, DC], XDT)
        xt = singles.tile([128, nkt, SA], XDT)
        # 3 issuing queues; xt arrives in k-tile pairs so the first (DR
        # pair-chained) projection starts on piece 0 while the rest stream
        nc.scalar.dma_start(wq_sb[:, :, :], wq_d[:, :, :])
        nc.sync.dma_start(xt[:, 0:2, :], xT_d[:, 0:2, :])
        nc.gpsimd.dma_start(wk_sb[:, :, :], wk_d[:, :, :])
        nc.sync.dma_start(xt[:, 2:4, :], xT_d[:, 2:4, :])
        nc.scalar.dma_start(xt[:, 4:6, :], xT_d[:, 4:6, :])
        nc.gpsimd.dma_start(xt[:, 6:8, :], xT_d[:, 6:8, :])
        nc.scalar.dma_start(wv_sb[:, :, :], wv_d[:, :, :])
        if Y_FP8:
            # [128, 2, D]: pair p rows as DR subtile p
            wo2 = singles.tile([128, 2, D], WODT, tag="wo2", name="wo2")
            nc.sync.dma_start(wo2[:, :, :],
                              wo_d[:, :].rearrange("(p k) d -> k p d", p=2))
            wo_sb = []
        else:
            wo_sb = []
            for p in range(HC // 2):
                w = singles.tile([128, D], DT, tag=f"wo{p}", name=f"wo{p}")
                nc.sync.dma_start(w, wo_d[p * 128:(p + 1) * 128, :])
                wo_sb.append(w)

        # ---- projections ----
        if SCORE_FP8:
            # [128, 2, SA]: head h on partitions 32h..32h+32, dim d of head h
            # at (p = 32h + d%32, j = d//32); W cols are host-permuted so the
            # m-th psum half is exactly the j=m slice.
            qt = singles.tile([128, 2, SA], SDT, tag="qt", name="qt")
            kt = singles.tile([128, 2, SA], SDT, tag="kt", name="kt")
        else:
            qt = [singles.tile([128, SA], DT, tag=f"qt{m}", name=f"qt{m}")
                  for m in range(2)]
            kt = [singles.tile([128, SA], DT, tag=f"kt{m}", name=f"kt{m}")
                  for m in range(2)]
        # 68 cols (V | ones | zero-pad): the dual-fp8 LDWEIGHTS subtile
        # stride (HC*cols) must be a multiple of 16, so cols % 4 == 0
        v_aug = singles.tile([128, nst, HC, 68], PDT)

        def proj_qk(m, dst, w_sb, q0, qn):
            ps = aux_ps.tile([128, 512], f32, tag="aux", name="ps")
            if PROJ_FP8:
                for t2 in range(nkt // 2):
                    nc.tensor.matmul(
                        ps[:, :qn],
                        w_sb[:, 2 * t2:2 * t2 + 2, m * 128:(m + 1) * 128],
                        xt[:, 2 * t2:2 * t2 + 2, q0:q0 + qn],
                        start=(t2 == 0), stop=(t2 == nkt // 2 - 1),
                        perf_mode=DR)
            else:
                for t in range(nkt):
                    nc.tensor.matmul(
                        ps[:, :qn],
                        mm(w_sb[:, t, m * 128:(m + 1) * 128]),
                        mm(xt[:, t, q0:q0 + qn]),
                        start=(t == 0), stop=(t == nkt - 1))
            if SCORE_FP8:
                # m-half m = heads (2m, 2m+1): psum [0:64] is their j=0
                # subtile, [64:128] their j=1 (cross-partition-base copy)
                nc.vector.tensor_copy(dst[64 * m:64 * m + 64, 0, q0:q0 + qn],
                                      ps[0:64, :qn])
                nc.vector.tensor_copy(dst[64 * m:64 * m + 64, 1, q0:q0 + qn],
                                      ps[64:128, :qn])
            else:
                nc.vector.tensor_copy(dst[m][:, q0:q0 + qn], ps[:, :qn])

        v_done = set()

        def proj_v(s):
            v_done.add(s)
            ps = aux_ps.tile([128, 512], f32, tag="aux", name="ps")
            if PROJ_FP8:
                for t2 in range(nkt // 2):
                    nc.tensor.matmul(
                        ps[:, :DC],
                        xt[:, 2 * t2:2 * t2 + 2, s * 128:(s + 1) * 128],
                        wv_sb[:, 2 * t2:2 * t2 + 2, :],
                        start=(t2 == 0), stop=(t2 == nkt // 2 - 1),
                        perf_mode=DR)
            else:
                for t in range(nkt):
                    nc.tensor.matmul(
                        ps[:, :DC],
                        mm(xt[:, t, s * 128:(s + 1) * 128]),
                        mm(wv_sb[:, t, :]),
                        start=(t == 0), stop=(t == nkt - 1))
            for h in range(HC):
                nc.vector.tensor_copy(v_aug[:, s, h, 0:64],
                                      ps[:, h * 64:(h + 1) * 64])

        # m=0 projections and the first V tiles run up-front (PE-dense, warms
        # HAM); remaining independent PE work (V tail, m=1 projections, Y of
        # finished chunks) is queued and drained between attention groups so
        # the PE never starves while ACT runs the exps.
        aux_jobs = []

        def drain_aux(k):
            for _ in range(min(k, len(aux_jobs))):
                aux_jobs.pop(0)()

        nc.vector.memset(v_aug[:, :, :, 64:65], 1.0)
        nc.vector.memset(v_aug[:, :, :, 65:68], 0.0)
        ebias = None
        if OT_FP8:
            # exp bias -ln16: pt = exp(s/8)/16 keeps exp under the fp8e4 clip
            ebias = singles.tile([128, 1], f32, tag="ebias", name="ebias")
            nc.vector.memset(ebias, -LN16)
        for (q0, qn) in qch:
            proj_qk(0, qt, wq_sb, q0, qn)
            proj_qk(0, kt, wk_sb, q0, qn)
        proj_v(0)
        proj_v(1)
        for s in range(2, nst):
            aux_jobs.append(lambda s=s: proj_v(s))
        for (q0, qn) in qch:
            aux_jobs.append(lambda a=q0, b=qn: proj_qk(1, qt, wq_sb, a, b))
            aux_jobs.append(lambda a=q0, b=qn: proj_qk(1, kt, wk_sb, a, b))

        # ---- attention + output projection, per q chunk ----
        for ci, (q0, qn) in enumerate(qch):
            ot_sb = [None] * HC
            osball = (otsb_pool.tile([128, 2, 512], fp8, tag="osball",
                                     name="osball") if Y_FP8 else None)
            for p in range(HC // 2):
                m = p
                if p == 1 and ci == 0:
                    # pair 1 needs the m=1 projections: pull them forward
                    drain_aux(len(aux_jobs))
                ot_ps = {}
                for h in (2 * p, 2 * p + 1):
                    ot_ps[h] = ot_ps_pool.tile([68, 512], f32, tag="ot",
                                               name="ot_ps")
                for si in range(0, nst, 2):
                    # NOTE: emission order is semantic order in Tile — a
                    # consumer emitted before its producer reads stale data.
                    # During chunk0/pair0 the V-projection jobs at the queue
                    # head MUST outpace the OT consumers: 2 jobs per si-group
                    # keeps proj_v(s) strictly ahead of OT reads of v_aug[s].
                    drain_aux(2 if (ci == 0 and p == 0) else 1)
                    npair = min(2, nst - si)
                    assert all(s in v_done for s in range(si, si + npair)), \
                        f"proj_v not emitted before OT consumer: {si}"

                    st, pt = {}, {}
                    for h in (2 * p, 2 * p + 1):
                        st[h] = mm_ps.tile([128, 2, 512], f32, tag="mm",
                                           name="st_ps")
                    for j in range(npair):
                        s = si + j
                        for h in (2 * p, 2 * p + 1):
                            if SCORE_FP8:
                                nc.tensor.matmul(
                                    st[h][:, j, :qn],
                                    kt[32 * h:32 * h + 32, :,
                                       s * 128:(s + 1) * 128],
                                    qt[32 * h:32 * h + 32, :, q0:q0 + qn],
                                    start=True, stop=True,
                                    perf_mode=DR,
                                    tile_position=(32 * h, 0))
                            else:
                                # adjacent (even,odd) matmuls with explicit
                                # row-group tile_position pack the PE array
                                r0 = (h % 2) * 64
                                nc.tensor.matmul(
                                    st[h][:, j, :qn],
                                    mm(kt[m][r0:r0 + 64,
                                             s * 128:(s + 1) * 128]),
                                    mm(qt[m][r0:r0 + 64, q0:q0 + qn]),
                                    start=True, stop=True,
                                    tile_position=(r0, 0))
                    for h in (2 * p, 2 * p + 1):
                        pt[h] = pt_pool.tile([128, 2, 512], PDT, tag="pt",
                                             name="pt")
                        nc.scalar.activation(
                            pt[h][:, :npair, :qn], st[h][:, :npair, :qn],
                            mybir.ActivationFunctionType.Exp,
                            scale=ESCALE,
                            bias=(ebias[:, :] if OT_FP8 else 0.0))
                    for h in (2 * p, 2 * p + 1):
                        if OT_FP8 and npair == 2:
                            nc.tensor.matmul(
                                ot_ps[h][:, :qn],
                                v_aug[:, si:si + 2, h, :],
                                pt[h][:, 0:2, :qn],
                                start=(si == 0), stop=(si + 1 == nst - 1),
                                perf_mode=DR)
                        else:
                            for j in range(npair):
                                s = si + j
                                nc.tensor.matmul(
                                    ot_ps[h][:, :qn],
                                    mm(v_aug[:, s, h, 0:68]) if not OT_FP8
                                    else v_aug[:, s, h, 0:68],
                                    mm(pt[h][:, j, :qn]) if not OT_FP8
                                    else pt[h][:, j, :qn],
                                    start=(s == 0), stop=(s == nst - 1))
                if Y_FP8:
                    osbp = osball[:, p, :]
                else:
                    osbp = otsb_pool.tile([128, 512], DT, tag=f"osbp{p}",
                                          name=f"osbp{p}")
                ot_sb[p] = osbp
                for h in (2 * p, 2 * p + 1):
                    # evacuate psum immediately (zt row + unscaled OT copy,
                    # both cheap DVE) so the bank frees for the next chunk's
                    # OT; the r chain (gpsimd-queue DMA hops) then runs off
                    # the critical path; osb is scaled before the Y matmuls.
                    r0 = 0 if h % 2 == 0 else 64
                    zt = zr_pool.tile([65, 512], f32, tag="zt", name="zt")
                    nc.vector.tensor_scalar(
                        out=zt[64:65, :qn], in0=ot_ps[h][64:65, :qn],
                        scalar1=CADD, scalar2=None, op0=mybir.AluOpType.add)
                    otu = otsb_pool.tile([128, 512], DT,
                                         tag=f"otu{h - 2 * p}", name="otu")
                    nc.vector.tensor_copy(otu[r0:r0 + 64, :qn],
                                          ot_ps[h][0:64, :qn])
                    # z [1,qn] -> [128,4] so the reciprocal runs wide on DVE;
                    # alternate issue queue by head parity so the two chains
                    # of a pair don't serialize on one sequencer at endgame
                    dq = nc.gpsimd if h % 2 == 0 else nc.sync
                    zq = zq_pool.tile([128, 4], f32, tag="zq", name="zq")
                    qp = (qn + 3) // 4
                    dq.dma_start(zq[:qp, :], zt[64:65, :qn])
                    nc.vector.reciprocal(zq[:qp, :], zq[:qp, :])
                    zd2 = zscr_pool.tile([1, 512], f32, tag="zd2", name="zd2")
                    dq.dma_start(zd2[0:1, :qn], zq[:qp, :])
                    rb = rbc_pool.tile([128, 512], f32, tag=f"rbc{h}",
                                       name=f"rbc{h}")
                    dq.dma_start(rb[r0:r0 + 64, :qn],
                                 zd2[0:1, :qn].to_broadcast((64, qn)))
                    nc.vector.scalar_tensor_tensor(
                        out=osbp[r0:r0 + 64, :qn],
                        in0=otu[r0:r0 + 64, :qn],
                        scalar=(1.0 / WSCALE if PROJ_FP8 else 1.0),
                        in1=rb[r0:r0 + 64, :qn],
                        op0=mybir.AluOpType.mult,
                        op1=mybir.AluOpType.mult)

            # output projection: osbp packs the head pair on 128 partitions,
            # so each pair is a single K=128 accumulating matmul per output
            # chunk.  p-outer/dch-inner keeps the stationary operand
            # back-to-back for weight-reuse.  Queued so the Y matmuls fill
            # PE bubbles of the next chunk's (ACT-bound) attention.
            def y_job(q0, qn, jt, osb_pair, osball=None):
                qtn = min(128, qn - jt * 128)
                yps = [aux_ps.tile([128, 512], f32, tag="aux", name=f"y{di}")
                       for di in range(len(dch))]
                for di, (d0, dn) in enumerate(dch):
                    if Y_FP8:
                        nc.tensor.matmul(
                            yps[di][:qtn, :dn],
                            osball[:, :, jt * 128:jt * 128 + qtn],
                            wo2[:, :, d0:d0 + dn],
                            start=True, stop=True, perf_mode=DR)
                    else:
                        for p in range(HC // 2):
                            nc.tensor.matmul(
                                yps[di][:qtn, :dn],
                                mm(osb_pair[p][:, jt * 128:jt * 128 + qtn]),
                                mm(wo_sb[p][:, d0:d0 + dn]),
                                start=(p == 0), stop=(p == HC // 2 - 1))
                for di, (d0, dn) in enumerate(dch):
                    yo = yout_pool.tile([128, 512], bf16, tag="yo", name="yo")
                    if Y_FP8:
                        nc.vector.tensor_scalar(
                            out=yo[:qtn, :dn], in0=yps[di][:qtn, :dn],
                            scalar1=1.0 / WSCALE, scalar2=None,
                            op0=mybir.AluOpType.mult)
                    else:
                        nc.vector.tensor_copy(yo[:qtn, :dn], yps[di][:qtn, :dn])
                    # scalar queue: the sync queue is congested with
                    # end-of-kernel semaphores right when the last y lands
                    nc.scalar.dma_start(
                        y_d[q0 + jt * 128: q0 + jt * 128 + qtn, d0:d0 + dn],
                        yo[:qtn, :dn])

            for jt in range((qn + 127) // 128):
                aux_jobs.append(
                    lambda a=q0, b=qn, j=jt, o=tuple(ot_sb[:HC // 2]),
                    ob=osball: y_job(a, b, j, o, ob))
        drain_aux(len(aux_jobs))
    nc.compile()
    return nc


_nc_cache: dict = {}


def _get_nc(SA: int):
    key = (SA, MM_DTYPE, SCORE_FP8, OT_FP8, PROJ_FP8, Y_FP8)
    if key not in _nc_cache:
        _nc_cache[key] = _build(SA, MM_DTYPE)
    return _nc_cache[key]


def _score_perm():
    """W column permutation (within each head-group's 256 cols): m-half m
    covers heads (2m, 2m+1); psum partitions [0:64] are their j=0 subtile
    (dims 0..31), [64:128] the j=1 subtile (dims 32..63), so head h sits at
    qt[32h:32h+32, j, :] with dim d = j*32 + p%32."""
    perm = np.empty(2 * 128, np.int64)
    for m in range(2):
        for p in range(128):
            h = 2 * m + (p % 64) // 32
            d = (p // 64) * 32 + (p % 32)
            perm[m * 128 + p] = h * 64 + d
    return perm


def _reference_fallback(x, gate, Wq, bq, Wk, bk, Wv, bv, Wo, bo):
    g = gate.astype(x.dtype)[..., None]
    q = (x @ Wq + bq) * g
    k = (x @ Wk + bk) * g
    v = (x @ Wv + bv) * g

    def split(t):
        return t.reshape(B, S, H, DH).transpose(0, 2, 1, 3)

    q, k, v = split(q), split(k), split(v)
    sc = np.einsum('bhqd,bhkd->bhqk', q, k) / np.float32(math.sqrt(DH))
    sc = sc - sc.max(axis=-1, keepdims=True)
    e = np.exp(sc)
    attn = e / e.sum(axis=-1, keepdims=True)
    out = np.einsum('bhqk,bhkd->bhqd', attn, v)
    out = out.transpose(0, 2, 1, 3).reshape(B, S, D)
    out = out @ Wo + bo
    return (x * (1.0 - g) + out * g).astype(np.float32)


def kernel(x, gate, Wq, bq, Wk, bk, Wv, bv, Wo, bo, _profile=None):
    x = np.asarray(x, np.float32)
    gate = np.asarray(gate)
    args = dict(x=x, gate=gate, Wq=np.asarray(Wq, np.float32),
                bq=np.asarray(bq, np.float32), Wk=np.asarray(Wk, np.float32),
                bk=np.asarray(bk, np.float32), Wv=np.asarray(Wv, np.float32),
                bv=np.asarray(bv, np.float32), Wo=np.asarray(Wo, np.float32),
                bo=np.asarray(bo, np.float32))

    def _shuf(a):
        Dd, C = a.shape
        return np.ascontiguousarray(
            a.reshape(Dd // 128, 128, C).transpose(1, 0, 2))

    idxs = [np.nonzero(gate[b])[0] for b in range(B)]
    n_act = [len(i) for i in idxs]
    # the compaction trick needs zero q/k/v biases and at least one active
    # and one inactive token per batch; otherwise fall back to exact numpy
    if (any(np.abs(args[k]).max() > 0 for k in ("bq", "bk", "bv"))
            or min(n_act) == 0 or max(n_act) == S):
        return _reference_fallback(**args)

    SA = ((max(n_act) + 127) // 128) * 128
    npdt = _NPDT[MM_DTYPE]
    xdt = ml_dtypes.float8_e4m3 if PROJ_FP8 else npdt
    ws = WSCALE if PROJ_FP8 else 1.0
    perm = _score_perm() if SCORE_FP8 else np.arange(256)

    in_maps = []
    for b in range(B):
        xa = np.zeros((SA, D), np.float32)
        xa[:n_act[b]] = x[b, idxs[b]]
        xT = _shuf(np.ascontiguousarray(xa.T).astype(xdt))
        for g in range(GROUPS):
            cs = slice(g * DC, (g + 1) * DC)
            in_maps.append({
                "xT": xT,
                "wq": _shuf((args["Wq"][:, cs][:, perm] * ws).astype(xdt)),
                "wk": _shuf((args["Wk"][:, cs][:, perm] * ws).astype(xdt)),
                "wv": _shuf((args["Wv"][:, cs] * ws).astype(xdt)),
                "wo": np.ascontiguousarray(
                    args["Wo"][cs, :] * (WSCALE if Y_FP8 else 1.0)).astype(
                    ml_dtypes.float8_e4m3 if Y_FP8 else npdt),
            })

    nc = _get_nc(SA)
    kw = dict(_profile) if _profile else {}
    kw.pop("result", None)
    res = run_bass_kernel_spmd(nc, in_maps, core_ids=list(range(NCORES)), **kw)
    if _profile is not None:
        _profile["result"] = res

    out = x.copy()
    for b in range(B):
        Y = np.zeros((SA, D), np.float32)
        for g in range(GROUPS):
            Y += res.results[b * GROUPS + g]["y"].astype(np.float32)
        out[b, idxs[b]] = Y[:n_act[b]] + args["bo"]
    return out



# revision 46
# speedup vs baseline: 1.2748x; 1.0080x over previous
"""Entropy-gated multi-head attention on 8 Trainium2 NeuronCores.

Sharding: core c = b*4 + g handles batch b (of 2) and head-group g (4 of the
16 heads).  Tokens with gate==0 pass x through untouched and contribute
exactly zero k/v (zero biases), so the device only processes the compacted
active tokens (~half), with the softmax denominator corrected by the count of
inactive tokens: each inactive key contributes exp(0)=1 to the softmax sum
(scores vs. zeroed k are exactly 0) and nothing to the numerator (v=0).

Device math per core (no max-subtraction; scores are O(5) so exp is safe):
  QT = Wq_g^T x^T, KT = Wk_g^T x^T           [256, SA]
  V  = x Wv_g                                 [SA, 256]
  per head h: PT = exp((KT_h^T QT_h)/8)/16    [SA_k, SA_q]  (fp8, DoubleRow)
              OT' = [V_h | 1]^T PT            [65, SA_q] (row 64 = colsum)
              r = 1/(Z + (S - SA)/16)         broadcast to [64, SA_q] via DMA
              osb_h = OT * r                  (scaled attention out, lhsT form)
  Y(q, :) = sum_h osb_h^T Wo_h                (K=128 pair-packed psum accum)
Host sums the 4 per-group partial Y per batch, adds bo, scatters into x.

fp8 path: scores contract DH=64 as [32 partitions x 2 k-subtiles] DoubleRow
(W columns host-permuted so QT/KT psum halves land directly in the
[128, 2, SA] layout); OT contracts key-tile pairs as [128, 2, 65] DoubleRow
with PT written by ACT as exp(s/8)/16 in fp8e4 (1/16 keeps exp under the
+-240 fp8e4 clip; the softmax correction uses CADD/16 to compensate).
"""

import math
from contextlib import ExitStack

import numpy as np
import ml_dtypes

import concourse.bass as bass
import concourse.mybir as mybir
from concourse import bacc
import concourse.tile as tile
from concourse.bass_utils import run_bass_kernel_spmd

B, S, D = 2, 2048, 1024
H, DH = 16, 64
NCORES = 8
GROUPS = NCORES // B          # head-groups per batch = 4
HC = H // GROUPS              # heads per core = 4
DC = HC * DH                  # head-group width = 256

# matmul operand dtype for projections/Y: "bf16" | "f32" | "f32r"
MM_DTYPE = "bf16"
SCORE_FP8 = False             # 32-row DR scores stream slower than bf16: keep bf16
OT_FP8 = True                 # fp8e4 pt/v_aug, DoubleRow over key-tile pairs
PROJ_FP8 = True               # fp8e4 x/Wq/Wk/Wv, DoubleRow over k-tile pairs
Y_FP8 = False                 # DR Y saved no wall time (fill-in work); bf16
# W host-scale 16 keeps fp8 weights in the normal range; compensated by
# exp-scale/256 (q,k both x16) and the osb output scale 1/16 (v x16)
WSCALE = 16.0

_DT = {
    "bf16": mybir.dt.bfloat16,
    "f32": mybir.dt.float32,
    "f32r": mybir.dt.float32,
}
_NPDT = {
    "bf16": ml_dtypes.bfloat16,
    "f32": np.float32,
    "f32r": np.float32,
}

f32 = mybir.dt.float32
bf16 = mybir.dt.bfloat16
fp8 = mybir.dt.float8e4
LN16 = float(np.log(16.0))
DR = mybir.MatmulPerfMode.DoubleRow


def _chunks(total, step):
    out = []
    o = 0
    while o < total:
        out.append((o, min(step, total - o)))
        o += step
    return out


def _build(SA: int, dtype_tag: str) -> bass.Bass:
    DT = _DT[dtype_tag]
    SDT = fp8 if SCORE_FP8 else DT   # qt/kt storage
    PDT = fp8 if OT_FP8 else DT      # pt/v_aug storage
    XDT = fp8 if PROJ_FP8 else DT    # x / Wq / Wk / Wv storage
    PT_SCALE = 16.0 if OT_FP8 else 1.0
    if PROJ_FP8:
        assert OT_FP8, "PROJ_FP8 requires the fp8 OT path (scale bookkeeping)"
    # with PROJ_FP8, q/k carry a WSCALE factor each -> scores are WSCALE^2 up
    ESCALE = 0.125 / (WSCALE * WSCALE if PROJ_FP8 else 1.0)

    def mm(ap):
        if dtype_tag == "f32r":
            return ap.bitcast(mybir.dt.float32r)
        return ap

    nkt = D // 128            # 8 contraction tiles for projections
    nst = SA // 128           # token tiles
    qch = _chunks(SA, 512)    # q chunks
    dch = _chunks(D, 512)     # output-dim chunks
    CADD = float(S - SA) / PT_SCALE

    nc = bacc.Bacc()
    # host pre-shuffles to [128, t, cols] so every input DMA is 128 big
    # contiguous descriptors (vs ~1k strided rows via rearrange)
    xT_d = nc.dram_tensor("xT", [128, nkt, SA], XDT, kind="ExternalInput")
    wq_d = nc.dram_tensor("wq", [128, nkt, DC], XDT, kind="ExternalInput")
    wk_d = nc.dram_tensor("wk", [128, nkt, DC], XDT, kind="ExternalInput")
    wv_d = nc.dram_tensor("wv", [128, nkt, DC], XDT, kind="ExternalInput")
    WODT = fp8 if Y_FP8 else DT
    wo_d = nc.dram_tensor("wo", [DC, D], WODT, kind="ExternalInput")
    y_d = nc.dram_tensor("y", [SA, D], bf16, kind="ExternalOutput")

    with tile.TileContext(nc) as tc, ExitStack() as ctx:
        singles = ctx.enter_context(tc.tile_pool(name="singles", bufs=1))
        pt_pool = ctx.enter_context(tc.tile_pool(name="pt", bufs=6))
        otsb_pool = ctx.enter_context(tc.tile_pool(name="otsb", bufs=2))
        zr_pool = ctx.enter_context(tc.tile_pool(name="zr", bufs=2))
        zq_pool = ctx.enter_context(tc.tile_pool(name="zq", bufs=2))
        rbc_pool = ctx.enter_context(tc.tile_pool(name="rbc", bufs=2))
        yout_pool = ctx.enter_context(tc.tile_pool(name="yout", bufs=3))
        zscr_pool = ctx.enter_context(tc.tile_pool(name="zscr", bufs=8,
                                                   space="DRAM"))
        # PSUM: st pairs 2x2 banks + aux (proj/V/Y) 2x1 + ot 2x1 = 8 banks
        mm_ps = ctx.enter_context(tc.tile_pool(name="mmps", bufs=2, space="PSUM"))
        aux_ps = ctx.enter_context(tc.tile_pool(name="auxps", bufs=2, space="PSUM"))
        ot_ps_pool = ctx.enter_context(tc.tile_pool(name="otps", bufs=2, space="PSUM"))

        # ---- persistent SBUF; one batched DMA per tensor/chunk keeps the
        # sync-queue issue time (~0.6us per dma_start) off the startup path
        wq_sb = singles.tile([128, nkt, DC], XDT)
        wk_sb = singles.tile([128, nkt, DC], XDT)
        wv_sb = singles.tile([128, nkt, DC], XDT)
        xt = singles.tile([128, nkt, SA], XDT)
        # 3 issuing queues; xt arrives in k-tile pairs so the first (DR
        # pair-chained) projection starts on piece 0 while the rest stream
        nc.scalar.dma_start(wq_sb[:, :, :], wq_d[:, :, :])
        nc.sync.dma_start(xt[:, 0:2, :], xT_d[:, 0:2, :])
        nc.gpsimd.dma_start(wk_sb[:, :, :], wk_d[:, :, :])
        nc.sync.dma_start(xt[:, 2:4, :], xT_d[:, 2:4, :])
        nc.scalar.dma_start(xt[:, 4:6, :], xT_d[:, 4:6, :])
        nc.gpsimd.dma_start(xt[:, 6:8, :], xT_d[:, 6:8, :])
        nc.scalar.dma_start(wv_sb[:, :, :], wv_d[:, :, :])
        if Y_FP8:
            # [128, 2, D]: pair p rows as DR subtile p
            wo2 = singles.tile([128, 2, D], WODT, tag="wo2", name="wo2")
            nc.sync.dma_start(wo2[:, :, :],
                              wo_d[:, :].rearrange("(p k) d -> k p d", p=2))
            wo_sb = []
        else:
            wo_sb = []
            for p in range(HC // 2):
                w = singles.tile([128, D], DT, tag=f"wo{p}", name=f"wo{p}")
                nc.sync.dma_start(w, wo_d[p * 128:(p + 1) * 128, :])
                wo_sb.append(w)

        # ---- projections ----
        if SCORE_FP8:
            # [128, 2, SA]: head h on partitions 32h..32h+32, dim d of head h
            # at (p = 32h + d%32, j = d//32); W cols are host-permuted so the
            # m-th psum half is exactly the j=m slice.
            qt = singles.tile([128, 2, SA], SDT, tag="qt", name="qt")
            kt = singles.tile([128, 2, SA], SDT, tag="kt", name="kt")
        else:
            qt = [singles.tile([128, SA], DT, tag=f"qt{m}", name=f"qt{m}")
                  for m in range(2)]
            kt = [singles.tile([128, SA], DT, tag=f"kt{m}", name=f"kt{m}")
                  for m in range(2)]
        # 68 cols (V | ones | zero-pad): the dual-fp8 LDWEIGHTS subtile
        # stride (HC*cols) must be a multiple of 16, so cols % 4 == 0
        v_aug = singles.tile([128, nst, HC, 68], PDT)

        def proj_qk(m, dst, w_sb, q0, qn):
            ps = aux_ps.tile([128, 512], f32, tag="aux", name="ps")
            if PROJ_FP8:
                for t2 in range(nkt // 2):
                    nc.tensor.matmul(
                        ps[:, :qn],
                        w_sb[:, 2 * t2:2 * t2 + 2, m * 128:(m + 1) * 128],
                        xt[:, 2 * t2:2 * t2 + 2, q0:q0 + qn],
                        start=(t2 == 0), stop=(t2 == nkt // 2 - 1),
                        perf_mode=DR)
            else:
                for t in range(nkt):
                    nc.tensor.matmul(
                        ps[:, :qn],
                        mm(w_sb[:, t, m * 128:(m + 1) * 128]),
                        mm(xt[:, t, q0:q0 + qn]),
                        start=(t == 0), stop=(t == nkt - 1))
            if SCORE_FP8:
                # m-half m = heads (2m, 2m+1): psum [0:64] is their j=0
                # subtile, [64:128] their j=1 (cross-partition-base copy)
                nc.vector.tensor_copy(dst[64 * m:64 * m + 64, 0, q0:q0 + qn],
                                      ps[0:64, :qn])
                nc.vector.tensor_copy(dst[64 * m:64 * m + 64, 1, q0:q0 + qn],
                                      ps[64:128, :qn])
            else:
                nc.vector.tensor_copy(dst[m][:, q0:q0 + qn], ps[:, :qn])

        v_done = set()

        def proj_v(s):
            v_done.add(s)
            ps = aux_ps.tile([128, 512], f32, tag="aux", name="ps")
            if PROJ_FP8:
                for t2 in range(nkt // 2):
                    nc.tensor.matmul(
                        ps[:, :DC],
                        xt[:, 2 * t2:2 * t2 + 2, s * 128:(s + 1) * 128],
                        wv_sb[:, 2 * t2:2 * t2 + 2, :],
                        start=(t2 == 0), stop=(t2 == nkt // 2 - 1),
                        perf_mode=DR)
            else:
                for t in range(nkt):
                    nc.tensor.matmul(
                        ps[:, :DC],
                        mm(xt[:, t, s * 128:(s + 1) * 128]),
                        mm(wv_sb[:, t, :]),
                        start=(t == 0), stop=(t == nkt - 1))
            for h in range(HC):
                nc.vector.tensor_copy(v_aug[:, s, h, 0:64],
                                      ps[:, h * 64:(h + 1) * 64])

        # m=0 projections and the first V tiles run up-front (PE-dense, warms
        # HAM); remaining independent PE work (V tail, m=1 projections, Y of
        # finished chunks) is queued and drained between attention groups so
        # the PE never starves while ACT runs the exps.
        aux_jobs = []

        def drain_aux(k):
            for _ in range(min(k, len(aux_jobs))):
                aux_jobs.pop(0)()

        nc.vector.memset(v_aug[:, :, :, 64:65], 1.0)
        nc.vector.memset(v_aug[:, :, :, 65:68], 0.0)
        ebias = None
        if OT_FP8:
            # exp bias -ln16: pt = exp(s/8)/16 keeps exp under the fp8e4 clip
            ebias = singles.tile([128, 1], f32, tag="ebias", name="ebias")
            nc.vector.memset(ebias, -LN16)
        for (q0, qn) in qch:
            proj_qk(0, qt, wq_sb, q0, qn)
            proj_qk(0, kt, wk_sb, q0, qn)
        proj_v(0)
        proj_v(1)
        for s in range(2, nst):
            aux_jobs.append(lambda s=s: proj_v(s))
        for (q0, qn) in qch:
            aux_jobs.append(lambda a=q0, b=qn: proj_qk(1, qt, wq_sb, a, b))
            aux_jobs.append(lambda a=q0, b=qn: proj_qk(1, kt, wk_sb, a, b))

        # ---- attention + output projection, per q chunk ----
        for ci, (q0, qn) in enumerate(qch):
            ot_sb = [None] * HC
            osball = (otsb_pool.tile([128, 2, 512], fp8, tag="osball",
                                     name="osball") if Y_FP8 else None)
            for p in range(HC // 2):
                m = p
                if p == 1 and ci == 0:
                    # pair 1 needs the m=1 projections: pull them forward
                    drain_aux(len(aux_jobs))
                ot_ps = {}
                for h in (2 * p, 2 * p + 1):
                    ot_ps[h] = ot_ps_pool.tile([68, 512], f32, tag="ot",
                                               name="ot_ps")
                for si in range(0, nst, 2):
                    # NOTE: emission order is semantic order in Tile — a
                    # consumer emitted before its producer reads stale data.
                    # During chunk0/pair0 the V-projection jobs at the queue
                    # head MUST outpace the OT consumers: 2 jobs per si-group
                    # keeps proj_v(s) strictly ahead of OT reads of v_aug[s].
                    drain_aux(2 if (ci == 0 and p == 0)
                              or ci == len(qch) - 1 else 1)
                    npair = min(2, nst - si)
                    assert all(s in v_done for s in range(si, si + npair)), \
                        f"proj_v not emitted before OT consumer: {si}"

                    st, pt = {}, {}
                    for h in (2 * p, 2 * p + 1):
                        st[h] = mm_ps.tile([128, 2, 512], f32, tag="mm",
                                           name="st_ps")
                    for j in range(npair):
                        s = si + j
                        for h in (2 * p, 2 * p + 1):
                            if SCORE_FP8:
                                nc.tensor.matmul(
                                    st[h][:, j, :qn],
                                    kt[32 * h:32 * h + 32, :,
                                       s * 128:(s + 1) * 128],
                                    qt[32 * h:32 * h + 32, :, q0:q0 + qn],
                                    start=True, stop=True,
                                    perf_mode=DR,
                                    tile_position=(32 * h, 0))
                            else:
                                # adjacent (even,odd) matmuls with explicit
                                # row-group tile_position pack the PE array
                                r0 = (h % 2) * 64
                                nc.tensor.matmul(
                                    st[h][:, j, :qn],
                                    mm(kt[m][r0:r0 + 64,
                                             s * 128:(s + 1) * 128]),
                                    mm(qt[m][r0:r0 + 64, q0:q0 + qn]),
                                    start=True, stop=True,
                                    tile_position=(r0, 0))
                    for h in (2 * p, 2 * p + 1):
                        pt[h] = pt_pool.tile([128, 2, 512], PDT, tag="pt",
                                             name="pt")
                        nc.scalar.activation(
                            pt[h][:, :npair, :qn], st[h][:, :npair, :qn],
                            mybir.ActivationFunctionType.Exp,
                            scale=ESCALE,
                            bias=(ebias[:, :] if OT_FP8 else 0.0))
                    for h in (2 * p, 2 * p + 1):
                        if OT_FP8 and npair == 2:
                            nc.tensor.matmul(
                                ot_ps[h][:, :qn],
                                v_aug[:, si:si + 2, h, :],
                                pt[h][:, 0:2, :qn],
                                start=(si == 0), stop=(si + 1 == nst - 1),
                                perf_mode=DR)
                        else:
                            for j in range(npair):
                                s = si + j
                                nc.tensor.matmul(
                                    ot_ps[h][:, :qn],
                                    mm(v_aug[:, s, h, 0:68]) if not OT_FP8
                                    else v_aug[:, s, h, 0:68],
                                    mm(pt[h][:, j, :qn]) if not OT_FP8
                                    else pt[h][:, j, :qn],
                                    start=(s == 0), stop=(s == nst - 1))
                if Y_FP8:
                    osbp = osball[:, p, :]
                else:
                    osbp = otsb_pool.tile([128, 512], DT, tag=f"osbp{p}",
                                          name=f"osbp{p}")
                ot_sb[p] = osbp
                for h in (2 * p, 2 * p + 1):
                    # evacuate psum immediately (zt row + unscaled OT copy,
                    # both cheap DVE) so the bank frees for the next chunk's
                    # OT; the r chain (gpsimd-queue DMA hops) then runs off
                    # the critical path; osb is scaled before the Y matmuls.
                    r0 = 0 if h % 2 == 0 else 64
                    zt = zr_pool.tile([65, 512], f32, tag="zt", name="zt")
                    nc.vector.tensor_scalar(
                        out=zt[64:65, :qn], in0=ot_ps[h][64:65, :qn],
                        scalar1=CADD, scalar2=None, op0=mybir.AluOpType.add)
                    otu = otsb_pool.tile([128, 512], DT,
                                         tag=f"otu{h - 2 * p}", name="otu")
                    nc.vector.tensor_copy(otu[r0:r0 + 64, :qn],
                                          ot_ps[h][0:64, :qn])
                    # z [1,qn] -> [128,4] so the reciprocal runs wide on DVE;
                    # alternate issue queue by head parity so the two chains
                    # of a pair don't serialize on one sequencer at endgame
                    dq = nc.scalar if h % 2 == 0 else nc.sync
                    zq = zq_pool.tile([128, 4], f32, tag="zq", name="zq")
                    qp = (qn + 3) // 4
                    dq.dma_start(zq[:qp, :], zt[64:65, :qn])
                    nc.vector.reciprocal(zq[:qp, :], zq[:qp, :])
                    zd2 = zscr_pool.tile([1, 512], f32, tag="zd2", name="zd2")
                    dq.dma_start(zd2[0:1, :qn], zq[:qp, :])
                    rb = rbc_pool.tile([128, 512], f32, tag=f"rbc{h}",
                                       name=f"rbc{h}")
                    dq.dma_start(rb[r0:r0 + 64, :qn],
                                 zd2[0:1, :qn].to_broadcast((64, qn)))
                    nc.vector.scalar_tensor_tensor(
                        out=osbp[r0:r0 + 64, :qn],
                        in0=otu[r0:r0 + 64, :qn],
                        scalar=(1.0 / WSCALE if PROJ_FP8 else 1.0),
                        in1=rb[r0:r0 + 64, :qn],
                        op0=mybir.AluOpType.mult,
                        op1=mybir.AluOpType.mult)

            # output projection: osbp packs the head pair on 128 partitions,
            # so each pair is a single K=128 accumulating matmul per output
            # chunk.  p-outer/dch-inner keeps the stationary operand
            # back-to-back for weight-reuse.  Queued so the Y matmuls fill
            # PE bubbles of the next chunk's (ACT-bound) attention.
            def y_job(q0, qn, jt, osb_pair, osball=None):
                qtn = min(128, qn - jt * 128)
                yps = [aux_ps.tile([128, 512], f32, tag="aux", name=f"y{di}")
                       for di in range(len(dch))]
                for di, (d0, dn) in enumerate(dch):
                    if Y_FP8:
                        nc.tensor.matmul(
                            yps[di][:qtn, :dn],
                            osball[:, :, jt * 128:jt * 128 + qtn],
                            wo2[:, :, d0:d0 + dn],
                            start=True, stop=True, perf_mode=DR)
                    else:
                        for p in range(HC // 2):
                            nc.tensor.matmul(
                                yps[di][:qtn, :dn],
                                mm(osb_pair[p][:, jt * 128:jt * 128 + qtn]),
                                mm(wo_sb[p][:, d0:d0 + dn]),
                                start=(p == 0), stop=(p == HC // 2 - 1))
                for di, (d0, dn) in enumerate(dch):
                    yo = yout_pool.tile([128, 512], bf16, tag="yo", name="yo")
                    if Y_FP8:
                        nc.vector.tensor_scalar(
                            out=yo[:qtn, :dn], in0=yps[di][:qtn, :dn],
                            scalar1=1.0 / WSCALE, scalar2=None,
                            op0=mybir.AluOpType.mult)
                    else:
                        nc.vector.tensor_copy(yo[:qtn, :dn], yps[di][:qtn, :dn])
                    # scalar queue: the sync queue is congested with
                    # end-of-kernel semaphores right when the last y lands
                    nc.scalar.dma_start(
                        y_d[q0 + jt * 128: q0 + jt * 128 + qtn, d0:d0 + dn],
                        yo[:qtn, :dn])

            for jt in range((qn + 127) // 128):
                aux_jobs.append(
                    lambda a=q0, b=qn, j=jt, o=tuple(ot_sb[:HC // 2]),
                    ob=osball: y_job(a, b, j, o, ob))
        drain_aux(len(aux_jobs))
    nc.compile()
    return nc


_nc_cache: dict = {}


def _get_nc(SA: int):
    key = (SA, MM_DTYPE, SCORE_FP8, OT_FP8, PROJ_FP8, Y_FP8)
    if key not in _nc_cache:
        _nc_cache[key] = _build(SA, MM_DTYPE)
    return _nc_cache[key]


def _score_perm():
    """W column permutation (within each head-group's 256 cols): m-half m
    covers heads (2m, 2m+1); psum partitions [0:64] are their j=0 subtile
    (dims 0..31), [64:128] the j=1 subtile (dims 32..63), so head h sits at
    qt[32h:32h+32, j, :] with dim d = j*32 + p%32."""
    perm = np.empty(2 * 128, np.int64)
    for m in range(2):
        for p in range(128):
            h = 2 * m + (p % 64) // 32
            d = (p // 64) * 32 + (p % 32)
            perm[m * 128 + p] = h * 64 + d
    return perm


def _reference_fallback(x, gate, Wq, bq, Wk, bk, Wv, bv, Wo, bo):
    g = gate.astype(x.dtype)[..., None]
    q = (x @ Wq + bq) * g
    k = (x @ Wk + bk) * g
    v = (x @ Wv + bv) * g

    def split(t):
        return t.reshape(B, S, H, DH).transpose(0, 2, 1, 3)

    q, k, v = split(q), split(k), split(v)
    sc = np.einsum('bhqd,bhkd->bhqk', q, k) / np.float32(math.sqrt(DH))
    sc = sc - sc.max(axis=-1, keepdims=True)
    e = np.exp(sc)
    attn = e / e.sum(axis=-1, keepdims=True)
    out = np.einsum('bhqk,bhkd->bhqd', attn, v)
    out = out.transpose(0, 2, 1, 3).reshape(B, S, D)
    out = out @ Wo + bo
    return (x * (1.0 - g) + out * g).astype(np.float32)


def kernel(x, gate, Wq, bq, Wk, bk, Wv, bv, Wo, bo, _profile=None):
    x = np.asarray(x, np.float32)
    gate = np.asarray(gate)
    args = dict(x=x, gate=gate, Wq=np.asarray(Wq, np.float32),
                bq=np.asarray(bq, np.float32), Wk=np.asarray(Wk, np.float32),
                bk=np.asarray(bk, np.float32), Wv=np.asarray(Wv, np.float32),
                bv=np.asarray(bv, np.float32), Wo=np.asarray(Wo, np.float32),
                bo=np.asarray(bo, np.float32))

    def _shuf(a):
        Dd, C = a.shape
        return np.ascontiguousarray(
            a.reshape(Dd // 128, 128, C).transpose(1, 0, 2))

    idxs = [np.nonzero(gate[b])[0] for b in range(B)]
    n_act = [len(i) for i in idxs]
    # the compaction trick needs zero q/k/v biases and at least one active
    # and one inactive token per batch; otherwise fall back to exact numpy
    if (any(np.abs(args[k]).max() > 0 for k in ("bq", "bk", "bv"))
            or min(n_act) == 0 or max(n_act) == S):
        return _reference_fallback(**args)

    SA = ((max(n_act) + 127) // 128) * 128
    npdt = _NPDT[MM_DTYPE]
    xdt = ml_dtypes.float8_e4m3 if PROJ_FP8 else npdt
    ws = WSCALE if PROJ_FP8 else 1.0
    perm = _score_perm() if SCORE_FP8 else np.arange(256)

    in_maps = []
    for b in range(B):
        xa = np.zeros((SA, D), np.float32)
        xa[:n_act[b]] = x[b, idxs[b]]
        xT = _shuf(np.ascontiguousarray(xa.T).astype(xdt))
        for g in range(GROUPS):
            cs = slice(g * DC, (g + 1) * DC)
            in_maps.append({
                "xT": xT,
                "wq": _shuf((args["Wq"][:, cs][:, perm] * ws).astype(xdt)),
                "wk": _shuf((args["Wk"][:, cs][:, perm] * ws).astype(xdt)),
                "wv": _shuf((args["Wv"][:, cs] * ws).astype(xdt)),
                "wo": np.ascontiguousarray(
                    args["Wo"][cs, :] * (WSCALE if Y_FP8 else 1.0)).astype(
                    ml_dtypes.float8_e4m3 if Y_FP8 else npdt),
            })

    nc = _get_nc(SA)
    kw = dict(_profile) if _profile else {}
    kw.pop("result", None)
    res = run_bass_kernel_spmd(nc, in_maps, core_ids=list(range(NCORES)), **kw)
    if _profile is not None:
        _profile["result"] = res

    out = x.copy()
    for b in range(B):
        Y = np.zeros((SA, D), np.float32)
        for g in range(GROUPS):
            Y += res.results[b * GROUPS + g]["y"].astype(np.float32)
        out[b, idxs[b]] = Y[:n_act[b]] + args["bo"]
    return out

